# revision 21
# baseline (speedup 1.0000x reference)
"""CDConvBlock Trainium2 kernel (8-core SPMD, data-parallel over batch).

Math (per image, x: [C, H*W] channel-major):
    offset = tanh(w_off @ x + b_off)                      [2, HW]
    gx = clip(gx_base + offset[0], -1, 1), gy likewise
    A' = (w_W @ w_pc) * w_dw[None, :]                     [C, C]
    out = bilinear_zeros(A' @ x, gx, gy) + b_W + x
The two 1x1 convs and the depthwise scale commute with the per-channel
bilinear gather, so they fold into the single matrix A' applied BEFORE
sampling.

v2 pipeline (channel-major output; per core = one image):
  1. Stream x (cast to bf16, channel-major, kept resident in SBUF).
     For each 128-pixel row chunk, an x-stationary matmul (lhsT = x
     chunk) against rhs = [A'.T | w_off.T] yields a PIXEL-major psum
     [128px, 258] = [y row | dx_pre | dy_pre]; y rows are cast to fp8
     tokens and written TWICE to DRAM in a duplicated layout
     A[j] = [y_j | y_{j+128}]  (so one gather element = 4 corners).
  2. Offset math on small [128x, 128y] tiles -> bilinear base index
     idx = by*128+bx and 4 edge-corrected weights packed k-minor into
     wq4[x, y, k] (zeros padding folded into the weights).
  3. ONE dma_gather per 8-row chunk fetches 1 KB elements
     [TL|BL|TR|BR] (elem_step=512B overlapping, elem_size=1024B).
  4. Bilinear combine on PE, channel-major: per output row, psum
     [ch_half, px] is seeded with x via an identity matmul, then 4
     matmuls accumulate  G_q^T @ diag(w_q)  (diag tiles built in ONE
     DVE op per row from a replicated-identity constant).  ACT drains
     psum + b_W (per-partition bias) to bf16; result streams to DRAM
     channel-major [C, HW] -- the residual never needs a transpose.
Host only shards/reshapes (batch split, weight transposes, constants).
"""

import numpy as np

import concourse.bass as bass
import concourse.bacc as bacc
import concourse.mybir as mybir
from concourse.bass_types import AP
from concourse.tile import TileContext
from concourse.tile_rust import add_dep_helper
from concourse import bass_utils
from concourse import library_config

F32 = mybir.dt.float32
BF16 = mybir.dt.bfloat16
I16 = mybir.dt.int16
FP8 = mybir.dt.float8e4
NP_BF16 = mybir.dt.np(BF16)
OP = mybir.AluOpType
AF = mybir.ActivationFunctionType

B, C, H, W = 8, 256, 128, 128
HW = H * W
N_CORES = 8

XB = 16     # image rows per x-stream batch
GC = 8      # image rows per gather chunk

_CACHE = {}


def _build(reps=1):
    nc = bacc.Bacc(
        "TRN2", target_bir_lowering=False, debug=False, num_devices=N_CORES
    )
    x_d = nc.dram_tensor("x", [C, HW], BF16, kind="ExternalInput")
    wpc_d = nc.dram_tensor("w_pc", [C, C], F32, kind="ExternalInput")
    wwt_d = nc.dram_tensor("w_W_T", [C, C], F32, kind="ExternalInput")
    wdw_d = nc.dram_tensor("w_dw_p", [128, 2], F32, kind="ExternalInput")
    wofft_d = nc.dram_tensor("w_off_T", [C, 2], F32, kind="ExternalInput")
    boff_d = nc.dram_tensor("b_off_b", [1, 2], F32, kind="ExternalInput")
    bwh_d = nc.dram_tensor("bW_h", [128, 2], F32, kind="ExternalInput")
    gxb_d = nc.dram_tensor("gxb", [W, 1], F32, kind="ExternalInput")
    gyb_d = nc.dram_tensor("gyb", [1, H], F32, kind="ExternalInput")
    i4t_d = nc.dram_tensor("i4t", [128, 512], F32, kind="ExternalInput")
    id_d = nc.dram_tensor("id128", [128, 128], F32, kind="ExternalInput")
    sel_d = nc.dram_tensor("sel", [128, 8 * 128], F32, kind="ExternalInput")
    out_d = nc.dram_tensor("out_cm", [C, HW], BF16, kind="ExternalOutput")

    with TileContext(nc) as tc:
        for _ in range(reps):
            _kernel_body(
                nc, tc, x_d, wpc_d, wwt_d, wdw_d, wofft_d, boff_d, bwh_d,
                gxb_d, gyb_d, i4t_d, id_d, sel_d, out_d,
            )
    nc.finalize()
    return nc


def _kernel_body(
    nc, tc, x_d, wpc_d, wwt_d, wdw_d, wofft_d, boff_d, bwh_d,
    gxb_d, gyb_d, i4t_d, id_d, sel_d, out_d,
):
    v = nc.vector
    g = nc.gpsimd
    s = nc.scalar

    with (
        tc.tile_pool(name="persist", bufs=1) as pp,
        tc.tile_pool(name="wload", bufs=1) as wp,
        tc.tile_pool(name="scr", bufs=2) as scr,
        tc.tile_pool(name="yst", bufs=2) as yp,
        tc.tile_pool(name="gat", bufs=3) as gp,
        tc.tile_pool(name="dga", bufs=8) as ap_,
        tc.tile_pool(name="ost", bufs=2) as op_,
        tc.tile_pool(name="psA", bufs=3, space="PSUM") as psa,
        tc.tile_pool(name="psD", bufs=3, space="PSUM") as psd,
        tc.tile_pool(name="psO", bufs=2, space="PSUM") as pso,
        tc.tile_pool(name="dram", bufs=1, space="DRAM") as dp,
    ):
        # gpsimd ucode library for DMAGatherAnt
        lib_load = g.load_library(library_config.mlp)

        # ---------------- Phase A: weight prep ----------------
        wpc_sb = [wp.tile([128, C], F32, tag=f"wpc{m}", name=f"wpc{m}") for m in range(2)]
        wwt_sb = [wp.tile([128, C], F32, tag=f"wwt{m}", name=f"wwt{m}") for m in range(2)]
        wdw_sb = wp.tile([128, 2], F32, tag="wdw", name="wdw")
        wofft_sb = [wp.tile([128, 2], F32, tag=f"woft{m}", name=f"woft{m}") for m in range(2)]
        for m in range(2):
            nc.sync.dma_start(wpc_sb[m][:], wpc_d.ap()[m * 128:(m + 1) * 128, :])
            nc.sync.dma_start(wwt_sb[m][:], wwt_d.ap()[m * 128:(m + 1) * 128, :])
            nc.sync.dma_start(
                wofft_sb[m][:], wofft_d.ap()[m * 128:(m + 1) * 128, :]
            )
        nc.sync.dma_start(wdw_sb[:], wdw_d.ap())

        boff_sb = pp.tile([128, 2], F32, tag="boff", name="boff")
        bwh_sb = pp.tile([128, 2], F32, tag="bwh", name="bwh")
        gxb_sb = pp.tile([128, 1], F32, tag="gxb", name="gxb")
        gyb_sb = pp.tile([128, H], F32, tag="gyb", name="gyb")
        nc.sync.dma_start(boff_sb[:], boff_d.ap().to_broadcast((128, 2)))
        nc.sync.dma_start(bwh_sb[:], bwh_d.ap())
        nc.sync.dma_start(gxb_sb[:], gxb_d.ap())
        nc.sync.dma_start(gyb_sb[:], gyb_d.ap().to_broadcast((128, H)))

        i4_sb = pp.tile([128, 128, 4], BF16, tag="i4", name="i4")
        g.dma_start(i4_sb[:].rearrange("p j k -> p (j k)"), i4t_d.ap())
        id_sb = pp.tile([128, 128], BF16, tag="id", name="id")
        g.dma_start(id_sb[:], id_d.ap())
        sel_sb = pp.tile([128, 8 * 128], F32, tag="sel", name="sel")
        nc.sync.dma_start(sel_sb[:], sel_d.ap())

        # persistent x (bf16, channel-major), 8 column-batches per half.
        # All loads issued up front so later y writes (whose deps resolve
        # mid-phase-B) never head-of-line-block them on the sync queue.
        n_xbat = H // XB
        x_sb = [
            pp.tile([128, 2, XB * 128], BF16, tag=f"x_{t}", name=f"x_{t}")
            for t in range(n_xbat)
        ]
        for t in range(n_xbat):
            cols = slice(t * XB * 128, (t + 1) * XB * 128)
            nc.sync.dma_start(
                x_sb[t][:],
                x_d.ap()[:, cols].rearrange("(h p) c -> p h c", h=2),
            )

        # B matrices: B[cb] = [A'.T | w_off.T] rows cb*128..cb*128+128
        B_sb = [pp.tile([128, C], BF16, tag=f"B{cb}", name=f"B{cb}") for cb in range(2)]
        woffb_sb = [
            pp.tile([128, 2], BF16, tag=f"wofb{m}", name=f"wofb{m}")
            for m in range(2)
        ]
        for cb in range(2):
            ps = psa.tile([128, 2, 256], F32, tag="psA", name="psA")
            for m in range(2):
                nc.tensor.matmul(
                    ps[:, 0, 0:C],
                    wpc_sb[m][:, cb * 128:(cb + 1) * 128],
                    wwt_sb[m][:],
                    start=(m == 0),
                    stop=(m == 1),
                )
            v.tensor_scalar(
                B_sb[cb][:, 0:C], ps[:, 0, 0:C], wdw_sb[:, cb:cb + 1], None,
                OP.mult,
            )
            v.tensor_copy(woffb_sb[cb][:], wofft_sb[cb][:])

        offs_sb = pp.tile([128, 2, H], F32, tag="offs", name="offs")    # [x, {dx,dy}, y]
        wq4 = pp.tile([128, H, 4], F32, tag="wq4", name="wq4")
        ibT = pp.tile([128, HW // 16], I16, tag="ibT", name="ibT")

        # y-dup token array: A[j] = [y_j | y_{j+128}], 512B rows
        y_dram = dp.tile([HW * 2 * C], FP8, tag="ydram", name="ydram")
        yv4 = y_dram[:].rearrange("(j d) -> j d", d=2 * C)
        # zero the never-read last image row so the finite-guard on the
        # gather's full address window passes
        zt = pp.tile([128, 2 * C], FP8, tag="zt", name="zt")
        v.memzero(zt[:])
        wz = nc.sync.dma_start(
            yv4[HW - 128:HW, :].rearrange("(r x) d -> x r d", x=128),
            zt[:].unsqueeze(1),
        )

        # ---------------- Phase C body (per y-column half) ----------------
        ibT_v = ibT[:].rearrange("p (y q) -> p y q", q=8)

        def axis_pipeline(o, base_is_tensor, ys, yn, eng=v):
            """offs[:, o, ys] -> (b (f32 col base), wA, wB) for one axis."""
            d = scr.tile([128, yn], F32, tag="sc_d", name="sc_d")
            if base_is_tensor:
                s.activation(d, offs_sb[:, o, ys], AF.Tanh, bias=boff_sb[:, o:o + 1])
                gg = scr.tile([128, yn], F32, tag="sc_g", name="sc_g")
                eng.scalar_tensor_tensor(gg, d, 1.0, gyb_sb[:, ys], OP.mult, OP.add)
            else:
                s.activation(d, offs_sb[:, o, ys], AF.Tanh, bias=boff_sb[:, o:o + 1])
                gg = scr.tile([128, yn], F32, tag="sc_g", name="sc_g")
                eng.tensor_scalar(gg, d, gxb_sb[:, 0:1], None, OP.add)
            # u = ix + 384 lives in the f32 binade [256, 512) where ulp is
            # 2^-15, so clearing the low 15 mantissa bits IS floor(u) for
            # the whole range ix in [-0.5, 127.5].  All exact.
            ix = scr.tile([128, yn], F32, tag="sc_ix", name="sc_ix")
            eng.tensor_scalar(ix, gg, -1.0, 1.0, OP.max, OP.min)
            eng.tensor_scalar(ix, ix, 64.0, 63.5 + 384.0, OP.mult, OP.add)
            ufl = scr.tile([128, yn], F32, tag="sc_ufl", name="sc_ufl")
            eng.tensor_scalar(
                ufl[:].bitcast(mybir.dt.uint32), ix[:].bitcast(mybir.dt.uint32),
                0xFFFF8000, None, OP.bitwise_and,
            )
            fx = scr.tile([128, yn], F32, tag="sc_fx", name="sc_fx")
            eng.tensor_tensor(fx, ix, ufl, OP.subtract)
            mneg = scr.tile([128, yn], F32, tag="sc_mneg", name="sc_mneg")
            eng.tensor_scalar(mneg, ufl, 383.5, None, OP.is_lt)
            mhi = scr.tile([128, yn], F32, tag="sc_mhi", name="sc_mhi")
            eng.tensor_scalar(mhi, ufl, 510.5, None, OP.is_gt)
            # clamped base, still in u-domain (bx + 384)
            bcol = scr.tile([128, yn], F32, tag="sc_b", name="sc_b")
            eng.tensor_scalar(bcol, ufl, 384.0, 510.0, OP.max, OP.min)
            # common = 1 - mneg - mhi;  g1 = 1 - fx
            cm = scr.tile([128, yn], F32, tag="sc_cm", name="sc_cm")
            eng.tensor_tensor(cm, mneg, mhi, OP.add)
            eng.tensor_scalar(cm, cm, -1.0, 1.0, OP.mult, OP.add)
            g1 = scr.tile([128, yn], F32, tag="sc_g1", name="sc_g1")
            eng.tensor_scalar(g1, fx, -1.0, 1.0, OP.mult, OP.add)
            # wA = g1*common + fx*mneg ; wB = fx*common + g1*mhi
            wA = scr.tile([128, yn], F32, tag="sc_wA", name="sc_wA")
            t1 = scr.tile([128, yn], F32, tag="sc_t1", name="sc_t1")
            eng.tensor_tensor(wA, g1, cm, OP.mult)
            eng.tensor_tensor(t1, fx, mneg, OP.mult)
            eng.tensor_tensor(wA, wA, t1, OP.add)
            eng.tensor_scalar(wA, wA, 0.25, None, OP.mult)
            wB_ = scr.tile([128, yn], F32, tag="sc_wB", name="sc_wB")
            eng.tensor_tensor(wB_, fx, cm, OP.mult)
            eng.tensor_tensor(t1, g1, mhi, OP.mult)
            eng.tensor_tensor(wB_, wB_, t1, OP.add)
            eng.tensor_scalar(wB_, wB_, 0.25, None, OP.mult)
            return bcol, wA, wB_

        idf_halves = {}

        def phase_c_dve(half):
            ys = slice(half * 64, (half + 1) * 64)
            yn = 64
            bx, wL, wR = axis_pipeline(0, False, ys, yn)
            by, wT, wB_ = axis_pipeline(1, True, ys, yn)
            # packed k-minor weights; element layout is [TL | BL | TR | BR]
            v.tensor_tensor(wq4[:, ys, 0], wT, wL, OP.mult)
            v.tensor_tensor(wq4[:, ys, 1], wB_, wL, OP.mult)
            v.tensor_tensor(wq4[:, ys, 2], wT, wR, OP.mult)
            v.tensor_tensor(wq4[:, ys, 3], wB_, wR, OP.mult)
            # token index (y-major rows): idx = by*128 + bx.
            # bx/by are still in the u-domain (+384 each): subtract 384*129.
            idf = scr.tile([128, yn], F32, tag=f"sc_idf{half}", name="sc_idf")
            v.scalar_tensor_tensor(idf, by, 128.0, bx, OP.mult, OP.add)
            v.tensor_scalar(idf, idf, -384.0 * 129.0, None, OP.add)
            idf_halves[half] = idf

        def phase_c_sel(half):
            # Fold [x, y] -> gather layout [j%16, j//16] (j = y*128+x) with
            # the replication across the 8 16-partition groups baked in:
            #   ibT[p, y*8+g] = idf[g*16 + p%16, y]
            # via 8 selector matmuls (SELg[x, p] = 1 iff x == g*16 + p%16,
            # f32 exact) + strided-destination cast copies.  Compute-engine
            # APs must start at 32-aligned partitions, so the 16-row fold
            # cannot be done with plain copies.  Issued only once idf is
            # ready so they never head-of-line-block the PE queue.
            ys = slice(half * 64, (half + 1) * 64)
            yn = 64
            idf = idf_halves[half]
            for q in range(8):
                psS = psa.tile([128, 2, 256], F32, tag="psA", name="psS")
                nc.tensor.matmul(
                    psS[:, 0, 0:yn],
                    sel_sb[:, q * 128:(q + 1) * 128],
                    idf[:],
                    start=True,
                    stop=True,
                )
                v.tensor_copy(ibT_v[:, ys, q], psS[:, 0, 0:yn])

        # ---------------- Phase B: matmuls, y tokens ----------------
        # y rows are staged per 16-row batch plus the next batch's first row,
        # so each A-row [y_j | y_{j+128}] is written as one 512B run via an
        # overlapping-source DMA (row r and r+1 read twice).
        y_writes = [None] * n_xbat
        prev_yst = None

        def issue_ywrite(t, yst_t, nrows):
            src = AP(
                yst_t[:].tensor, yst_t[:].offset,
                [[(XB + 1) * C, 128], [C, nrows], [C, 2], [1, C]],
            )
            dst = yv4[t * XB * 128:(t * XB + nrows) * 128, :].rearrange(
                "(r x) d -> x r d", x=128
            )
            y_writes[t] = nc.sync.dma_start(dst, src)

        for t in range(n_xbat):
            yst = yp.tile([128, XB + 1, C], FP8, tag="yst", name="yst")
            psO = pso.tile([128, XB, 2], F32, tag="psO", name="psO")
            for hb in range(XB // 2):
                psA = psa.tile([128, 2, 256], F32, tag="psA", name="psA")
                for r in range(2):
                    cc = (hb * 2 + r) * 128
                    rr2 = hb * 2 + r
                    nc.tensor.matmul(
                        psA[:, r, 0:C], x_sb[t][:, 0, cc:cc + 128],
                        B_sb[0][:], start=True, stop=False,
                    )
                    nc.tensor.matmul(
                        psA[:, r, 0:C], x_sb[t][:, 1, cc:cc + 128],
                        B_sb[1][:], start=False, stop=True,
                    )
                    # raw offsets accumulate in a batch-persistent psum so
                    # they drain once per batch instead of once per pair
                    nc.tensor.matmul(
                        psO[:, rr2, :], x_sb[t][:, 0, cc:cc + 128],
                        woffb_sb[0][:], start=True, stop=False,
                        skip_group_check=True,
                    )
                    nc.tensor.matmul(
                        psO[:, rr2, :], x_sb[t][:, 1, cc:cc + 128],
                        woffb_sb[1][:], start=False, stop=True,
                        skip_group_check=True,
                    )
                rr = hb * 2
                # y tokens -> fp8 staging (ACT)
                s.copy(yst[:, rr:rr + 2, :], psA[:, :, 0:C])
                if hb == 0 and t > 0:
                    # previous batch needs this row as its 17th token row
                    s.copy(prev_yst[:, XB:XB + 1, :], psA[:, 0:1, 0:C])
                    issue_ywrite(t - 1, prev_yst, XB)
            # offsets: [x, r, o] -> offs[x, o, 16t+r], one drain per batch
            v.tensor_copy(
                offs_sb[:, :, t * XB:(t + 1) * XB].transpose([0, 2, 1]),
                psO[:],
            )
            prev_yst = yst
            if t == 3:
                phase_c_dve(0)
            elif t == 5:
                phase_c_sel(0)
        # last batch: 15 full A-rows (row 127's A-row is never gathered)
        issue_ywrite(n_xbat - 1, prev_yst, XB - 1)
        phase_c_dve(1)
        phase_c_sel(1)

        # ---------------- Phase D: gather + bilinear combine ----------------
        y2d = AP(y_dram[:].tensor, y_dram[:].offset, [[2 * C, HW - 1], [1, 4 * C]])
        n_chunk = H // GC
        nidx = GC * 128
        for k in range(n_chunk):
            g4 = gp.tile([128, GC, 4 * C], FP8, tag="g4", name="g4")
            icols = slice(k * nidx // 16, (k + 1) * nidx // 16)
            gi = g.dma_gather(
                g4[:], y2d, ibT[:, icols], nidx, nidx,
                elem_size=4 * C, elem_step=2 * C,
            )
            add_dep_helper(gi.ins, lib_load.ins, reason="gather needs mlp lib")
            add_dep_helper(gi.ins, wz.ins, reason="gather window covers tail")
            bmax = min(n_xbat - 1, (GC * k + GC - 1 + 65) // XB)
            for wy in y_writes[:bmax + 1]:
                add_dep_helper(gi.ins, wy.ins, reason="gather reads y_dram")
            ost = [
                op_.tile([128, GC, 128], BF16, tag=f"ost{h}", name=f"ost{h}")
                for h in range(2)
            ]
            # diag tiles for all 4 corners of each row, built in one DVE op
            dgas = []
            for r in range(GC):
                y = k * GC + r
                dga = ap_.tile([128, 128, 4], BF16, tag="dga", name="dga")
                v.tensor_tensor(
                    dga[:], i4_sb[:],
                    wq4[:, y, :].unsqueeze(1).broadcast_to((128, 128, 4)),
                    OP.mult,
                )
                dgas.append(dga)
            for grp in range(GC // 2):
                psD = psd.tile([128, 2, 2, 128], F32, tag="psD", name="psD")
                row0 = k * GC + grp * 2
                t = row0 // XB
                coff = (row0 % XB) * 128
                # seed: psum = x for both channel halves in ONE matmul --
                # psum "start" must cover the whole bank exactly once
                nc.tensor.matmul(
                    psD[:].rearrange("p h r x -> p (h r x)"),
                    id_sb[:],
                    x_sb[t][:, :, coff:coff + 256],
                    start=True, stop=False, skip_group_check=True,
                )
                for r2 in range(2):
                    r = grp * 2 + r2
                    dga = dgas[r]
                    for h in range(2):
                        for q in range(4):
                            nc.tensor.matmul(
                                psD[:, h, r2, :],
                                g4[:, r, q * 256 + h * 128: q * 256 + h * 128 + 128],
                                dga[:, :, q],
                                start=False,
                                stop=(q == 3),
                                skip_group_check=True,
                            )
                for h in range(2):
                    s.activation(
                        ost[h][:, grp * 2:grp * 2 + 2, :],
                        psD[:, h, :, :],
                        AF.Identity,
                        bias=bwh_sb[:, h:h + 1],
                    )
            for h in range(2):
                ov = out_d.ap()[
                    h * 128:(h + 1) * 128, k * nidx:(k + 1) * nidx
                ].rearrange("p (r x) -> p r x", x=128)
                nc.sync.dma_start(ov, ost[h][:])


def _sel_const():
    sel = np.zeros((128, 8, 128), dtype=np.float32)
    for gq in range(8):
        for p in range(128):
            sel[gq * 16 + p % 16, gq, p] = 1.0
    return sel.reshape(128, 8 * 128)


def _host_inputs(inputs):
    """Per-core in_maps from the full problem inputs (layout/shard only)."""
    x = np.asarray(inputs["x"], dtype=np.float32)
    w_dw = np.asarray(inputs["w_dw"], dtype=np.float32)
    w_off = np.asarray(inputs["w_off"], dtype=np.float32)
    b_off = np.asarray(inputs["b_off"], dtype=np.float32)
    w_pc = np.asarray(inputs["w_pc"], dtype=np.float32)
    w_W = np.asarray(inputs["w_W"], dtype=np.float32)
    b_W = np.asarray(inputs["b_W"], dtype=np.float32)

    lin_w = np.linspace(-1.0, 1.0, W, dtype=np.float32)
    lin_h = np.linspace(-1.0, 1.0, H, dtype=np.float32)
    ident = np.eye(128, dtype=np.float32)
    shared = {
        "w_pc": np.ascontiguousarray(w_pc),
        "w_W_T": np.ascontiguousarray(w_W.T),
        "w_dw_p": np.ascontiguousarray(w_dw.reshape(2, 128).T) * 16.0,
        "w_off_T": np.ascontiguousarray(w_off.T),
        "b_off_b": b_off.reshape(1, 2),
        "bW_h": np.ascontiguousarray(b_W.reshape(2, 128).T),
        "gxb": lin_w.reshape(W, 1),
        "gyb": lin_h.reshape(1, H),
        "i4t": np.ascontiguousarray(np.repeat(ident, 4, axis=1)),
        "id128": ident,
        "sel": _sel_const(),
    }
    in_maps = []
    for b in range(B):
        m = dict(shared)
        m["x"] = np.ascontiguousarray(x[b].reshape(C, HW)).astype(NP_BF16)
        in_maps.append(m)
    return in_maps


def postprocess_core(raw):
    """Device out_cm [C, HW] bf16 -> [C, H, W] f32."""
    return np.asarray(raw).astype(np.float32).reshape(C, H, W)


def kernel_with_results(trace=False, **inputs):
    if "nc" not in _CACHE:
        _CACHE["nc"] = _build()
    nc = _CACHE["nc"]
    in_maps = _host_inputs(inputs)
    res = bass_utils.run_bass_kernel_spmd(
        nc, in_maps, core_ids=list(range(N_CORES)), trace=trace
    )
    outs = [postprocess_core(res.results[b]["out_cm"]) for b in range(B)]
    return np.stack(outs, axis=0), res


def kernel(**inputs) -> np.ndarray:
    out, _ = kernel_with_results(**inputs)
    return out


# revision 23
# speedup vs baseline: 1.1102x; 1.1102x over previous
"""CDConvBlock Trainium2 kernel (8-core SPMD, data-parallel over batch).

Math (per image, x: [C, H*W] channel-major):
    offset = tanh(w_off @ x + b_off)                      [2, HW]
    gx = clip(gx_base + offset[0], -1, 1), gy likewise
    A' = (w_W @ w_pc) * w_dw[None, :]                     [C, C]
    out = bilinear_zeros(A' @ x, gx, gy) + b_W + x
The two 1x1 convs and the depthwise scale commute with the per-channel
bilinear gather, so they fold into the single matrix A' applied BEFORE
sampling.

v2 pipeline (channel-major output; per core = one image):
  1. Stream x (cast to bf16, channel-major, kept resident in SBUF).
     For each 128-pixel row chunk, an x-stationary matmul (lhsT = x
     chunk) against rhs = [A'.T | w_off.T] yields a PIXEL-major psum
     [128px, 258] = [y row | dx_pre | dy_pre]; y rows are cast to fp8
     tokens and written TWICE to DRAM in a duplicated layout
     A[j] = [y_j | y_{j+128}]  (so one gather element = 4 corners).
  2. Offset math on small [128x, 128y] tiles -> bilinear base index
     idx = by*128+bx and 4 edge-corrected weights packed k-minor into
     wq4[x, y, k] (zeros padding folded into the weights).
  3. ONE dma_gather per 8-row chunk fetches 1 KB elements
     [TL|BL|TR|BR] (elem_step=512B overlapping, elem_size=1024B).
  4. Bilinear combine on PE, channel-major: per output row, psum
     [ch_half, px] is seeded with x via an identity matmul, then 4
     matmuls accumulate  G_q^T @ diag(w_q)  (diag tiles built in ONE
     DVE op per row from a replicated-identity constant).  ACT drains
     psum + b_W (per-partition bias) to bf16; result streams to DRAM
     channel-major [C, HW] -- the residual never needs a transpose.
Host only shards/reshapes (batch split, weight transposes, constants).
"""

import numpy as np

import concourse.bass as bass
import concourse.bacc as bacc
import concourse.mybir as mybir
from concourse.bass_types import AP
from concourse.tile import TileContext
from concourse.tile_rust import add_dep_helper
from concourse import bass_utils
from concourse import library_config

F32 = mybir.dt.float32
BF16 = mybir.dt.bfloat16
I16 = mybir.dt.int16
FP8 = mybir.dt.float8e4
NP_BF16 = mybir.dt.np(BF16)
OP = mybir.AluOpType
AF = mybir.ActivationFunctionType

B, C, H, W = 8, 256, 128, 128
HW = H * W
N_CORES = 8

XB = 16     # image rows per x-stream batch
GC = 8      # image rows per gather chunk

_CACHE = {}


def _build(reps=1):
    nc = bacc.Bacc(
        "TRN2", target_bir_lowering=False, debug=False, num_devices=N_CORES
    )
    x_d = nc.dram_tensor("x", [C, HW], BF16, kind="ExternalInput")
    wpc_d = nc.dram_tensor("w_pc", [C, C], F32, kind="ExternalInput")
    wwt_d = nc.dram_tensor("w_W_T", [C, C], F32, kind="ExternalInput")
    wdw_d = nc.dram_tensor("w_dw_p", [128, 2], F32, kind="ExternalInput")
    wofft_d = nc.dram_tensor("w_off_T", [C, 2], F32, kind="ExternalInput")
    boff_d = nc.dram_tensor("b_off_b", [1, 2], F32, kind="ExternalInput")
    bwh_d = nc.dram_tensor("bW_h", [128, 2], F32, kind="ExternalInput")
    gxb_d = nc.dram_tensor("gxb", [W, 1], F32, kind="ExternalInput")
    gyb_d = nc.dram_tensor("gyb", [1, H], F32, kind="ExternalInput")
    i4t_d = nc.dram_tensor("i4t", [128, 512], F32, kind="ExternalInput")
    id_d = nc.dram_tensor("id128", [128, 128], F32, kind="ExternalInput")
    sel_d = nc.dram_tensor("sel", [128, 8 * 128], F32, kind="ExternalInput")
    out_d = nc.dram_tensor("out_cm", [C, HW], BF16, kind="ExternalOutput")

    with TileContext(nc) as tc:
        for _ in range(reps):
            _kernel_body(
                nc, tc, x_d, wpc_d, wwt_d, wdw_d, wofft_d, boff_d, bwh_d,
                gxb_d, gyb_d, i4t_d, id_d, sel_d, out_d,
            )
    nc.finalize()
    return nc


def _kernel_body(
    nc, tc, x_d, wpc_d, wwt_d, wdw_d, wofft_d, boff_d, bwh_d,
    gxb_d, gyb_d, i4t_d, id_d, sel_d, out_d,
):
    v = nc.vector
    g = nc.gpsimd
    s = nc.scalar

    with (
        tc.tile_pool(name="persist", bufs=1) as pp,
        tc.tile_pool(name="wload", bufs=1) as wp,
        tc.tile_pool(name="scr", bufs=2) as scr,
        tc.tile_pool(name="yst", bufs=2) as yp,
        tc.tile_pool(name="gat", bufs=3) as gp,
        tc.tile_pool(name="dga", bufs=8) as ap_,
        tc.tile_pool(name="ost", bufs=2) as op_,
        tc.tile_pool(name="psA", bufs=2, space="PSUM") as psa,
        tc.tile_pool(name="psD", bufs=2, space="PSUM") as psd,
        tc.tile_pool(name="psO", bufs=2, space="PSUM") as pso,
        tc.tile_pool(name="dram", bufs=1, space="DRAM") as dp,
    ):
        # gpsimd ucode library for DMAGatherAnt
        lib_load = g.load_library(library_config.mlp)

        # ---------------- Phase A: weight prep ----------------
        wpc_sb = [wp.tile([128, C], F32, tag=f"wpc{m}", name=f"wpc{m}") for m in range(2)]
        wwt_sb = [wp.tile([128, C], F32, tag=f"wwt{m}", name=f"wwt{m}") for m in range(2)]
        wdw_sb = wp.tile([128, 2], F32, tag="wdw", name="wdw")
        wofft_sb = [wp.tile([128, 2], F32, tag=f"woft{m}", name=f"woft{m}") for m in range(2)]
        for m in range(2):
            nc.sync.dma_start(wpc_sb[m][:], wpc_d.ap()[m * 128:(m + 1) * 128, :])
            nc.sync.dma_start(wwt_sb[m][:], wwt_d.ap()[m * 128:(m + 1) * 128, :])
            nc.sync.dma_start(
                wofft_sb[m][:], wofft_d.ap()[m * 128:(m + 1) * 128, :]
            )
        nc.sync.dma_start(wdw_sb[:], wdw_d.ap())

        boff_sb = pp.tile([128, 2], F32, tag="boff", name="boff")
        bwh_sb = pp.tile([128, 2], F32, tag="bwh", name="bwh")
        gxb_sb = pp.tile([128, 1], F32, tag="gxb", name="gxb")
        gyb_sb = pp.tile([128, H], F32, tag="gyb", name="gyb")
        nc.sync.dma_start(boff_sb[:], boff_d.ap().to_broadcast((128, 2)))
        nc.sync.dma_start(bwh_sb[:], bwh_d.ap())
        nc.sync.dma_start(gxb_sb[:], gxb_d.ap())
        nc.sync.dma_start(gyb_sb[:], gyb_d.ap().to_broadcast((128, H)))

        i4_sb = pp.tile([128, 128, 4], BF16, tag="i4", name="i4")
        g.dma_start(i4_sb[:].rearrange("p j k -> p (j k)"), i4t_d.ap())
        id_sb = pp.tile([128, 128], BF16, tag="id", name="id")
        g.dma_start(id_sb[:], id_d.ap())
        sel_sb = pp.tile([128, 8 * 128], F32, tag="sel", name="sel")
        nc.sync.dma_start(sel_sb[:], sel_d.ap())

        # persistent x (bf16, channel-major), 8 column-batches per half.
        # All loads issued up front so later y writes (whose deps resolve
        # mid-phase-B) never head-of-line-block them on the sync queue.
        n_xbat = H // XB
        x_sb = [
            pp.tile([128, 2, XB * 128], BF16, tag=f"x_{t}", name=f"x_{t}")
            for t in range(n_xbat)
        ]
        for t in range(n_xbat):
            cols = slice(t * XB * 128, (t + 1) * XB * 128)
            nc.sync.dma_start(
                x_sb[t][:],
                x_d.ap()[:, cols].rearrange("(h p) c -> p h c", h=2),
            )

        # B matrices: B[cb] = [A'.T | w_off.T] rows cb*128..cb*128+128
        B_sb = [pp.tile([128, C], BF16, tag=f"B{cb}", name=f"B{cb}") for cb in range(2)]
        woffb_sb = [
            pp.tile([128, 2], BF16, tag=f"wofb{m}", name=f"wofb{m}")
            for m in range(2)
        ]
        for cb in range(2):
            ps = psa.tile([128, 2, 256], F32, tag="psA", name="psA")
            for m in range(2):
                nc.tensor.matmul(
                    ps[:, 0, 0:C],
                    wpc_sb[m][:, cb * 128:(cb + 1) * 128],
                    wwt_sb[m][:],
                    start=(m == 0),
                    stop=(m == 1),
                )
            v.tensor_scalar(
                B_sb[cb][:, 0:C], ps[:, 0, 0:C], wdw_sb[:, cb:cb + 1], None,
                OP.mult,
            )
            v.tensor_copy(woffb_sb[cb][:], wofft_sb[cb][:])

        offs_sb = pp.tile([128, 2, H], F32, tag="offs", name="offs")    # [x, {dx,dy}, y]
        wq4 = pp.tile([128, H, 4], F32, tag="wq4", name="wq4")
        ibT = pp.tile([128, HW // 16], I16, tag="ibT", name="ibT")

        # y-dup token array: A[j] = [y_j | y_{j+128}], 512B rows
        y_dram = dp.tile([HW * 2 * C], FP8, tag="ydram", name="ydram")
        yv4 = y_dram[:].rearrange("(j d) -> j d", d=2 * C)
        # zero the never-read last image row so the finite-guard on the
        # gather's full address window passes
        zt = pp.tile([128, 2 * C], FP8, tag="zt", name="zt")
        v.memzero(zt[:])
        wz = nc.sync.dma_start(
            yv4[HW - 128:HW, :].rearrange("(r x) d -> x r d", x=128),
            zt[:].unsqueeze(1),
        )

        # ---------------- Phase C body (per y-column half) ----------------
        ibT_v = ibT[:].rearrange("p (y q) -> p y q", q=8)

        def axis_pipeline(o, base_is_tensor, ys, yn, eng=v):
            """offs[:, o, ys] -> (b (f32 col base), wA, wB) for one axis."""
            d = scr.tile([128, yn], F32, tag="sc_d", name="sc_d")
            if base_is_tensor:
                s.activation(d, offs_sb[:, o, ys], AF.Tanh, bias=boff_sb[:, o:o + 1])
                gg = scr.tile([128, yn], F32, tag="sc_g", name="sc_g")
                eng.scalar_tensor_tensor(gg, d, 1.0, gyb_sb[:, ys], OP.mult, OP.add)
            else:
                s.activation(d, offs_sb[:, o, ys], AF.Tanh, bias=boff_sb[:, o:o + 1])
                gg = scr.tile([128, yn], F32, tag="sc_g", name="sc_g")
                eng.tensor_scalar(gg, d, gxb_sb[:, 0:1], None, OP.add)
            # u = ix + 384 lives in the f32 binade [256, 512) where ulp is
            # 2^-15, so clearing the low 15 mantissa bits IS floor(u) for
            # the whole range ix in [-0.5, 127.5].  All exact.
            ix = scr.tile([128, yn], F32, tag="sc_ix", name="sc_ix")
            eng.tensor_scalar(ix, gg, -1.0, 1.0, OP.max, OP.min)
            eng.tensor_scalar(ix, ix, 64.0, 63.5 + 384.0, OP.mult, OP.add)
            ufl = scr.tile([128, yn], F32, tag="sc_ufl", name="sc_ufl")
            eng.tensor_scalar(
                ufl[:].bitcast(mybir.dt.uint32), ix[:].bitcast(mybir.dt.uint32),
                0xFFFF8000, None, OP.bitwise_and,
            )
            fx = scr.tile([128, yn], F32, tag="sc_fx", name="sc_fx")
            eng.tensor_tensor(fx, ix, ufl, OP.subtract)
            mneg = scr.tile([128, yn], F32, tag="sc_mneg", name="sc_mneg")
            eng.tensor_scalar(mneg, ufl, 383.5, None, OP.is_lt)
            mhi = scr.tile([128, yn], F32, tag="sc_mhi", name="sc_mhi")
            eng.tensor_scalar(mhi, ufl, 510.5, None, OP.is_gt)
            # clamped base, still in u-domain (bx + 384)
            bcol = scr.tile([128, yn], F32, tag="sc_b", name="sc_b")
            eng.tensor_scalar(bcol, ufl, 384.0, 510.0, OP.max, OP.min)
            # common = 1 - mneg - mhi;  g1 = 1 - fx
            cm = scr.tile([128, yn], F32, tag="sc_cm", name="sc_cm")
            eng.tensor_tensor(cm, mneg, mhi, OP.add)
            eng.tensor_scalar(cm, cm, -1.0, 1.0, OP.mult, OP.add)
            g1 = scr.tile([128, yn], F32, tag="sc_g1", name="sc_g1")
            eng.tensor_scalar(g1, fx, -1.0, 1.0, OP.mult, OP.add)
            # wA = g1*common + fx*mneg ; wB = fx*common + g1*mhi
            wA = scr.tile([128, yn], F32, tag="sc_wA", name="sc_wA")
            t1 = scr.tile([128, yn], F32, tag="sc_t1", name="sc_t1")
            eng.tensor_tensor(wA, g1, cm, OP.mult)
            eng.tensor_tensor(t1, fx, mneg, OP.mult)
            eng.tensor_tensor(wA, wA, t1, OP.add)
            eng.tensor_scalar(wA, wA, 0.25, None, OP.mult)
            wB_ = scr.tile([128, yn], F32, tag="sc_wB", name="sc_wB")
            eng.tensor_tensor(wB_, fx, cm, OP.mult)
            eng.tensor_tensor(t1, g1, mhi, OP.mult)
            eng.tensor_tensor(wB_, wB_, t1, OP.add)
            eng.tensor_scalar(wB_, wB_, 0.25, None, OP.mult)
            return bcol, wA, wB_

        idf_halves = {}

        def phase_c_dve(half):
            ys = slice(half * 64, (half + 1) * 64)
            yn = 64
            bx, wL, wR = axis_pipeline(0, False, ys, yn)
            by, wT, wB_ = axis_pipeline(1, True, ys, yn)
            # packed k-minor weights; element layout is [TL | BL | TR | BR]
            v.tensor_tensor(wq4[:, ys, 0], wT, wL, OP.mult)
            v.tensor_tensor(wq4[:, ys, 1], wB_, wL, OP.mult)
            v.tensor_tensor(wq4[:, ys, 2], wT, wR, OP.mult)
            v.tensor_tensor(wq4[:, ys, 3], wB_, wR, OP.mult)
            # token index (y-major rows): idx = by*128 + bx.
            # bx/by are still in the u-domain (+384 each): subtract 384*129.
            idf = scr.tile([128, yn], F32, tag=f"sc_idf{half}", name="sc_idf")
            v.scalar_tensor_tensor(idf, by, 128.0, bx, OP.mult, OP.add)
            v.tensor_scalar(idf, idf, -384.0 * 129.0, None, OP.add)
            idf_halves[half] = idf

        def phase_c_sel(half):
            # Fold [x, y] -> gather layout [j%16, j//16] (j = y*128+x) with
            # the replication across the 8 16-partition groups baked in:
            #   ibT[p, y*8+g] = idf[g*16 + p%16, y]
            # via 8 selector matmuls (SELg[x, p] = 1 iff x == g*16 + p%16,
            # f32 exact) + strided-destination cast copies.  Compute-engine
            # APs must start at 32-aligned partitions, so the 16-row fold
            # cannot be done with plain copies.  Issued only once idf is
            # ready so they never head-of-line-block the PE queue.
            ys = slice(half * 64, (half + 1) * 64)
            yn = 64
            idf = idf_halves[half]
            for q in range(8):
                psS = psa.tile([128, 2, 256], F32, tag="psA", name="psS")
                nc.tensor.matmul(
                    psS[:, 0, 0:yn],
                    sel_sb[:, q * 128:(q + 1) * 128],
                    idf[:],
                    start=True,
                    stop=True,
                )
                v.tensor_copy(ibT_v[:, ys, q], psS[:, 0, 0:yn])

        # ---------------- Phase B: matmuls, y tokens ----------------
        # y rows are staged per 16-row batch plus the next batch's first row,
        # so each A-row [y_j | y_{j+128}] is written as one 512B run via an
        # overlapping-source DMA (row r and r+1 read twice).
        y_writes = [None] * n_xbat
        prev_yst = None

        def issue_ywrite(t, yst_t, nrows):
            src = AP(
                yst_t[:].tensor, yst_t[:].offset,
                [[(XB + 1) * C, 128], [C, nrows], [C, 2], [1, C]],
            )
            dst = yv4[t * XB * 128:(t * XB + nrows) * 128, :].rearrange(
                "(r x) d -> x r d", x=128
            )
            y_writes[t] = nc.sync.dma_start(dst, src)

        for t in range(n_xbat):
            yst = yp.tile([128, XB + 1, C], FP8, tag="yst", name="yst")
            psO = pso.tile([128, XB, 2], F32, tag="psO", name="psO")
            for hb in range(XB // 2):
                psA = psa.tile([128, 2, 256], F32, tag="psA", name="psA")
                for r in range(2):
                    cc = (hb * 2 + r) * 128
                    rr2 = hb * 2 + r
                    nc.tensor.matmul(
                        psA[:, r, 0:C], x_sb[t][:, 0, cc:cc + 128],
                        B_sb[0][:], start=True, stop=False,
                    )
                    nc.tensor.matmul(
                        psA[:, r, 0:C], x_sb[t][:, 1, cc:cc + 128],
                        B_sb[1][:], start=False, stop=True,
                    )
                    # raw offsets accumulate in a batch-persistent psum so
                    # they drain once per batch instead of once per pair
                    nc.tensor.matmul(
                        psO[:, rr2, :], x_sb[t][:, 0, cc:cc + 128],
                        woffb_sb[0][:], start=True, stop=False,
                        skip_group_check=True,
                    )
                    nc.tensor.matmul(
                        psO[:, rr2, :], x_sb[t][:, 1, cc:cc + 128],
                        woffb_sb[1][:], start=False, stop=True,
                        skip_group_check=True,
                    )
                rr = hb * 2
                # y tokens -> fp8 staging (ACT)
                s.copy(yst[:, rr:rr + 2, :], psA[:, :, 0:C])
                if hb == 0 and t > 0:
                    # previous batch needs this row as its 17th token row
                    s.copy(prev_yst[:, XB:XB + 1, :], psA[:, 0:1, 0:C])
                    issue_ywrite(t - 1, prev_yst, XB)
            # offsets: [x, r, o] -> offs[x, o, 16t+r], one drain per batch
            v.tensor_copy(
                offs_sb[:, :, t * XB:(t + 1) * XB].transpose([0, 2, 1]),
                psO[:],
            )
            prev_yst = yst
            if t == 3:
                phase_c_dve(0)
            elif t == 5:
                phase_c_sel(0)
        # last batch: 15 full A-rows (row 127's A-row is never gathered)
        issue_ywrite(n_xbat - 1, prev_yst, XB - 1)
        phase_c_dve(1)
        phase_c_sel(1)

        # ---------------- Phase D: gather + bilinear combine ----------------
        y2d = AP(y_dram[:].tensor, y_dram[:].offset, [[2 * C, HW - 1], [1, 4 * C]])
        n_chunk = H // GC
        nidx = GC * 128
        for k in range(n_chunk):
            g4 = gp.tile([128, GC, 4 * C], FP8, tag="g4", name="g4")
            icols = slice(k * nidx // 16, (k + 1) * nidx // 16)
            gi = g.dma_gather(
                g4[:], y2d, ibT[:, icols], nidx, nidx,
                elem_size=4 * C, elem_step=2 * C,
            )
            add_dep_helper(gi.ins, lib_load.ins, reason="gather needs mlp lib")
            add_dep_helper(gi.ins, wz.ins, reason="gather window covers tail")
            bmax = min(n_xbat - 1, (GC * k + GC - 1 + 65) // XB)
            for wy in y_writes[:bmax + 1]:
                add_dep_helper(gi.ins, wy.ins, reason="gather reads y_dram")
            ost = [
                op_.tile([128, GC, 128], BF16, tag=f"ost{h}", name=f"ost{h}")
                for h in range(2)
            ]
            # diag tiles for all 4 corners of each row, built in one DVE op
            dgas = []
            for r in range(GC):
                y = k * GC + r
                dga = ap_.tile([128, 128, 4], BF16, tag="dga", name="dga")
                v.tensor_tensor(
                    dga[:], i4_sb[:],
                    wq4[:, y, :].unsqueeze(1).broadcast_to((128, 128, 4)),
                    OP.mult,
                )
                dgas.append(dga)
            for grp in range(GC // 4):
                psD = psd.tile([128, 2, 4, 128], F32, tag="psD", name="psD")
                row0 = k * GC + grp * 4
                t = row0 // XB
                coff = (row0 % XB) * 128
                for h in range(2):
                    # seed: psum = x; each seed covers its full psum bank
                    # exactly once (one open accumulation group per bank)
                    nc.tensor.matmul(
                        psD[:, h, :, :].rearrange("p r x -> p (r x)"),
                        id_sb[:],
                        x_sb[t][:, h, coff:coff + 512],
                        start=True, stop=False, skip_group_check=True,
                    )
                for r2 in range(4):
                    r = grp * 4 + r2
                    dga = dgas[r]
                    for h in range(2):
                        for q in range(4):
                            nc.tensor.matmul(
                                psD[:, h, r2, :],
                                g4[:, r, q * 256 + h * 128: q * 256 + h * 128 + 128],
                                dga[:, :, q],
                                start=False,
                                stop=(q == 3),
                                skip_group_check=True,
                            )
                for h in range(2):
                    s.activation(
                        ost[h][:, grp * 4:grp * 4 + 4, :],
                        psD[:, h, :, :],
                        AF.Identity,
                        bias=bwh_sb[:, h:h + 1],
                    )
            for h in range(2):
                ov = out_d.ap()[
                    h * 128:(h + 1) * 128, k * nidx:(k + 1) * nidx
                ].rearrange("p (r x) -> p r x", x=128)
                nc.sync.dma_start(ov, ost[h][:])


def _sel_const():
    sel = np.zeros((128, 8, 128), dtype=np.float32)
    for gq in range(8):
        for p in range(128):
            sel[gq * 16 + p % 16, gq, p] = 1.0
    return sel.reshape(128, 8 * 128)


def _host_inputs(inputs):
    """Per-core in_maps from the full problem inputs (layout/shard only)."""
    x = np.asarray(inputs["x"], dtype=np.float32)
    w_dw = np.asarray(inputs["w_dw"], dtype=np.float32)
    w_off = np.asarray(inputs["w_off"], dtype=np.float32)
    b_off = np.asarray(inputs["b_off"], dtype=np.float32)
    w_pc = np.asarray(inputs["w_pc"], dtype=np.float32)
    w_W = np.asarray(inputs["w_W"], dtype=np.float32)
    b_W = np.asarray(inputs["b_W"], dtype=np.float32)

    lin_w = np.linspace(-1.0, 1.0, W, dtype=np.float32)
    lin_h = np.linspace(-1.0, 1.0, H, dtype=np.float32)
    ident = np.eye(128, dtype=np.float32)
    shared = {
        "w_pc": np.ascontiguousarray(w_pc),
        "w_W_T": np.ascontiguousarray(w_W.T),
        "w_dw_p": np.ascontiguousarray(w_dw.reshape(2, 128).T) * 16.0,
        "w_off_T": np.ascontiguousarray(w_off.T),
        "b_off_b": b_off.reshape(1, 2),
        "bW_h": np.ascontiguousarray(b_W.reshape(2, 128).T),
        "gxb": lin_w.reshape(W, 1),
        "gyb": lin_h.reshape(1, H),
        "i4t": np.ascontiguousarray(np.repeat(ident, 4, axis=1)),
        "id128": ident,
        "sel": _sel_const(),
    }
    in_maps = []
    for b in range(B):
        m = dict(shared)
        m["x"] = np.ascontiguousarray(x[b].reshape(C, HW)).astype(NP_BF16)
        in_maps.append(m)
    return in_maps


def postprocess_core(raw):
    """Device out_cm [C, HW] bf16 -> [C, H, W] f32."""
    return np.asarray(raw).astype(np.float32).reshape(C, H, W)


def kernel_with_results(trace=False, **inputs):
    if "nc" not in _CACHE:
        _CACHE["nc"] = _build()
    nc = _CACHE["nc"]
    in_maps = _host_inputs(inputs)
    res = bass_utils.run_bass_kernel_spmd(
        nc, in_maps, core_ids=list(range(N_CORES)), trace=trace
    )
    outs = [postprocess_core(res.results[b]["out_cm"]) for b in range(B)]
    return np.stack(outs, axis=0), res


def kernel(**inputs) -> np.ndarray:
    out, _ = kernel_with_results(**inputs)
    return out


# revision 24
# speedup vs baseline: 1.2299x; 1.1078x over previous
"""CDConvBlock Trainium2 kernel (8-core SPMD, data-parallel over batch).

Math (per image, x: [C, H*W] channel-major):
    offset = tanh(w_off @ x + b_off)                      [2, HW]
    gx = clip(gx_base + offset[0], -1, 1), gy likewise
    A' = (w_W @ w_pc) * w_dw[None, :]                     [C, C]
    out = bilinear_zeros(A' @ x, gx, gy) + b_W + x
The two 1x1 convs and the depthwise scale commute with the per-channel
bilinear gather, so they fold into the single matrix A' applied BEFORE
sampling.

v2 pipeline (channel-major output; per core = one image):
  1. Stream x (cast to bf16, channel-major, kept resident in SBUF).
     For each 128-pixel row chunk, an x-stationary matmul (lhsT = x
     chunk) against rhs = [A'.T | w_off.T] yields a PIXEL-major psum
     [128px, 258] = [y row | dx_pre | dy_pre]; y rows are cast to fp8
     tokens and written TWICE to DRAM in a duplicated layout
     A[j] = [y_j | y_{j+128}]  (so one gather element = 4 corners).
  2. Offset math on small [128x, 128y] tiles -> bilinear base index
     idx = by*128+bx and 4 edge-corrected weights packed k-minor into
     wq4[x, y, k] (zeros padding folded into the weights).
  3. ONE dma_gather per 8-row chunk fetches 1 KB elements
     [TL|BL|TR|BR] (elem_step=512B overlapping, elem_size=1024B).
  4. Bilinear combine on PE, channel-major: per output row, psum
     [ch_half, px] is seeded with x via an identity matmul, then 4
     matmuls accumulate  G_q^T @ diag(w_q)  (diag tiles built in ONE
     DVE op per row from a replicated-identity constant).  ACT drains
     psum + b_W (per-partition bias) to bf16; result streams to DRAM
     channel-major [C, HW] -- the residual never needs a transpose.
Host only shards/reshapes (batch split, weight transposes, constants).
"""

import numpy as np

import concourse.bass as bass
import concourse.bacc as bacc
import concourse.mybir as mybir
from concourse.bass_types import AP
from concourse.tile import TileContext
from concourse.tile_rust import add_dep_helper
from concourse import bass_utils
from concourse import library_config

F32 = mybir.dt.float32
BF16 = mybir.dt.bfloat16
I16 = mybir.dt.int16
FP8 = mybir.dt.float8e4
NP_BF16 = mybir.dt.np(BF16)
OP = mybir.AluOpType
AF = mybir.ActivationFunctionType

B, C, H, W = 8, 256, 128, 128
HW = H * W
N_CORES = 8

XB = 16     # image rows per x-stream batch
GC = 8      # image rows per gather chunk

_CACHE = {}


def _build(reps=1):
    nc = bacc.Bacc(
        "TRN2", target_bir_lowering=False, debug=False, num_devices=N_CORES
    )
    x_d = nc.dram_tensor("x", [C, HW], BF16, kind="ExternalInput")
    wpc_d = nc.dram_tensor("w_pc", [C, C], F32, kind="ExternalInput")
    wwt_d = nc.dram_tensor("w_W_T", [C, C], F32, kind="ExternalInput")
    wdw_d = nc.dram_tensor("w_dw_p", [128, 2], F32, kind="ExternalInput")
    wofft_d = nc.dram_tensor("w_off_T", [C, 2], F32, kind="ExternalInput")
    boff_d = nc.dram_tensor("b_off_b", [1, 2], F32, kind="ExternalInput")
    bwh_d = nc.dram_tensor("bW_h", [128, 2], F32, kind="ExternalInput")
    gxb_d = nc.dram_tensor("gxb", [W, 1], F32, kind="ExternalInput")
    gyb_d = nc.dram_tensor("gyb", [1, H], F32, kind="ExternalInput")
    i4t_d = nc.dram_tensor("i4t", [128, 512], F32, kind="ExternalInput")
    id_d = nc.dram_tensor("id128", [128, 128], F32, kind="ExternalInput")
    sel_d = nc.dram_tensor("sel", [128, 8 * 128], F32, kind="ExternalInput")
    out_d = nc.dram_tensor("out_cm", [C, HW], BF16, kind="ExternalOutput")

    with TileContext(nc) as tc:
        for _ in range(reps):
            _kernel_body(
                nc, tc, x_d, wpc_d, wwt_d, wdw_d, wofft_d, boff_d, bwh_d,
                gxb_d, gyb_d, i4t_d, id_d, sel_d, out_d,
            )
    nc.finalize()
    return nc


def _kernel_body(
    nc, tc, x_d, wpc_d, wwt_d, wdw_d, wofft_d, boff_d, bwh_d,
    gxb_d, gyb_d, i4t_d, id_d, sel_d, out_d,
):
    v = nc.vector
    g = nc.gpsimd
    s = nc.scalar

    with (
        tc.tile_pool(name="persist", bufs=1) as pp,
        tc.tile_pool(name="wload", bufs=1) as wp,
        tc.tile_pool(name="scr", bufs=2) as scr,
        tc.tile_pool(name="yst", bufs=3) as yp,
        tc.tile_pool(name="gat", bufs=3) as gp,
        tc.tile_pool(name="dga", bufs=8) as ap_,
        tc.tile_pool(name="ost", bufs=2) as op_,
        tc.tile_pool(name="psA", bufs=2, space="PSUM") as psa,
        tc.tile_pool(name="psD", bufs=2, space="PSUM") as psd,
        tc.tile_pool(name="psO", bufs=2, space="PSUM") as pso,
        tc.tile_pool(name="dram", bufs=1, space="DRAM") as dp,
    ):
        # gpsimd ucode library for DMAGatherAnt
        lib_load = g.load_library(library_config.mlp)

        # ---------------- Phase A: weight prep ----------------
        wpc_sb = [wp.tile([128, C], F32, tag=f"wpc{m}", name=f"wpc{m}") for m in range(2)]
        wwt_sb = [wp.tile([128, C], F32, tag=f"wwt{m}", name=f"wwt{m}") for m in range(2)]
        wdw_sb = wp.tile([128, 2], F32, tag="wdw", name="wdw")
        wofft_sb = [wp.tile([128, 2], F32, tag=f"woft{m}", name=f"woft{m}") for m in range(2)]
        for m in range(2):
            nc.sync.dma_start(wpc_sb[m][:], wpc_d.ap()[m * 128:(m + 1) * 128, :])
            nc.sync.dma_start(wwt_sb[m][:], wwt_d.ap()[m * 128:(m + 1) * 128, :])
            nc.sync.dma_start(
                wofft_sb[m][:], wofft_d.ap()[m * 128:(m + 1) * 128, :]
            )
        nc.sync.dma_start(wdw_sb[:], wdw_d.ap())

        boff_sb = pp.tile([128, 2], F32, tag="boff", name="boff")
        bwh_sb = pp.tile([128, 2], F32, tag="bwh", name="bwh")
        gxb_sb = pp.tile([128, 1], F32, tag="gxb", name="gxb")
        gyb_sb = pp.tile([128, H], F32, tag="gyb", name="gyb")
        nc.sync.dma_start(boff_sb[:], boff_d.ap().to_broadcast((128, 2)))
        nc.sync.dma_start(bwh_sb[:], bwh_d.ap())
        nc.sync.dma_start(gxb_sb[:], gxb_d.ap())
        nc.sync.dma_start(gyb_sb[:], gyb_d.ap().to_broadcast((128, H)))

        i4_sb = pp.tile([128, 128, 4], BF16, tag="i4", name="i4")
        g.dma_start(i4_sb[:].rearrange("p j k -> p (j k)"), i4t_d.ap())
        id_sb = pp.tile([128, 128], BF16, tag="id", name="id")
        g.dma_start(id_sb[:], id_d.ap())
        sel_sb = pp.tile([128, 8 * 128], F32, tag="sel", name="sel")
        nc.sync.dma_start(sel_sb[:], sel_d.ap())

        # persistent x (bf16, channel-major), 8 column-batches per half.
        # All loads issued up front so later y writes (whose deps resolve
        # mid-phase-B) never head-of-line-block them on the sync queue.
        n_xbat = H // XB
        x_sb = [
            pp.tile([128, 2, XB * 128], BF16, tag=f"x_{t}", name=f"x_{t}")
            for t in range(n_xbat)
        ]
        for t in range(n_xbat):
            cols = slice(t * XB * 128, (t + 1) * XB * 128)
            nc.sync.dma_start(
                x_sb[t][:],
                x_d.ap()[:, cols].rearrange("(h p) c -> p h c", h=2),
            )

        # B matrices: B[cb] = [A'.T | w_off.T] rows cb*128..cb*128+128
        B_sb = [pp.tile([128, C], BF16, tag=f"B{cb}", name=f"B{cb}") for cb in range(2)]
        woffb_sb = [
            pp.tile([128, 2], BF16, tag=f"wofb{m}", name=f"wofb{m}")
            for m in range(2)
        ]
        for cb in range(2):
            ps = psa.tile([128, 2, 256], F32, tag="psA", name="psA")
            for m in range(2):
                nc.tensor.matmul(
                    ps[:, 0, 0:C],
                    wpc_sb[m][:, cb * 128:(cb + 1) * 128],
                    wwt_sb[m][:],
                    start=(m == 0),
                    stop=(m == 1),
                )
            v.tensor_scalar(
                B_sb[cb][:, 0:C], ps[:, 0, 0:C], wdw_sb[:, cb:cb + 1], None,
                OP.mult,
            )
            v.tensor_copy(woffb_sb[cb][:], wofft_sb[cb][:])

        offs_sb = pp.tile([128, 2, H], F32, tag="offs", name="offs")    # [x, {dx,dy}, y]
        wq4 = pp.tile([128, H, 4], F32, tag="wq4", name="wq4")
        ibT = pp.tile([128, HW // 16], I16, tag="ibT", name="ibT")

        # y-dup token array: A[j] = [y_j | y_{j+128}], 512B rows
        y_dram = dp.tile([HW * 2 * C], FP8, tag="ydram", name="ydram")
        yv4 = y_dram[:].rearrange("(j d) -> j d", d=2 * C)
        # zero the never-read last image row so the finite-guard on the
        # gather's full address window passes
        zt = pp.tile([128, 2 * C], FP8, tag="zt", name="zt")
        v.memzero(zt[:])
        wz = nc.sync.dma_start(
            yv4[HW - 128:HW, :].rearrange("(r x) d -> x r d", x=128),
            zt[:].unsqueeze(1),
        )

        # ---------------- Phase C body (per y-column half) ----------------
        ibT_v = ibT[:].rearrange("p (y q) -> p y q", q=8)

        def axis_pipeline(o, base_is_tensor, ys, yn, eng=v):
            """offs[:, o, ys] -> (b (f32 col base), wA, wB) for one axis."""
            d = scr.tile([128, yn], F32, tag="sc_d", name="sc_d")
            if base_is_tensor:
                s.activation(d, offs_sb[:, o, ys], AF.Tanh, bias=boff_sb[:, o:o + 1])
                gg = scr.tile([128, yn], F32, tag="sc_g", name="sc_g")
                eng.scalar_tensor_tensor(gg, d, 1.0, gyb_sb[:, ys], OP.mult, OP.add)
            else:
                s.activation(d, offs_sb[:, o, ys], AF.Tanh, bias=boff_sb[:, o:o + 1])
                gg = scr.tile([128, yn], F32, tag="sc_g", name="sc_g")
                eng.tensor_scalar(gg, d, gxb_sb[:, 0:1], None, OP.add)
            # u = ix + 384 lives in the f32 binade [256, 512) where ulp is
            # 2^-15, so clearing the low 15 mantissa bits IS floor(u) for
            # the whole range ix in [-0.5, 127.5].  All exact.
            ix = scr.tile([128, yn], F32, tag="sc_ix", name="sc_ix")
            eng.tensor_scalar(ix, gg, -1.0, 1.0, OP.max, OP.min)
            eng.tensor_scalar(ix, ix, 64.0, 63.5 + 384.0, OP.mult, OP.add)
            ufl = scr.tile([128, yn], F32, tag="sc_ufl", name="sc_ufl")
            eng.tensor_scalar(
                ufl[:].bitcast(mybir.dt.uint32), ix[:].bitcast(mybir.dt.uint32),
                0xFFFF8000, None, OP.bitwise_and,
            )
            fx = scr.tile([128, yn], F32, tag="sc_fx", name="sc_fx")
            eng.tensor_tensor(fx, ix, ufl, OP.subtract)
            mneg = scr.tile([128, yn], F32, tag="sc_mneg", name="sc_mneg")
            eng.tensor_scalar(mneg, ufl, 383.5, None, OP.is_lt)
            mhi = scr.tile([128, yn], F32, tag="sc_mhi", name="sc_mhi")
            eng.tensor_scalar(mhi, ufl, 510.5, None, OP.is_gt)
            # clamped base, still in u-domain (bx + 384)
            bcol = scr.tile([128, yn], F32, tag="sc_b", name="sc_b")
            eng.tensor_scalar(bcol, ufl, 384.0, 510.0, OP.max, OP.min)
            # common = 1 - mneg - mhi;  g1 = 1 - fx
            cm = scr.tile([128, yn], F32, tag="sc_cm", name="sc_cm")
            eng.tensor_tensor(cm, mneg, mhi, OP.add)
            eng.tensor_scalar(cm, cm, -1.0, 1.0, OP.mult, OP.add)
            g1 = scr.tile([128, yn], F32, tag="sc_g1", name="sc_g1")
            eng.tensor_scalar(g1, fx, -1.0, 1.0, OP.mult, OP.add)
            # wA = g1*common + fx*mneg ; wB = fx*common + g1*mhi
            wA = scr.tile([128, yn], F32, tag="sc_wA", name="sc_wA")
            t1 = scr.tile([128, yn], F32, tag="sc_t1", name="sc_t1")
            eng.tensor_tensor(wA, g1, cm, OP.mult)
            eng.tensor_tensor(t1, fx, mneg, OP.mult)
            eng.tensor_tensor(wA, wA, t1, OP.add)
            eng.tensor_scalar(wA, wA, 0.25, None, OP.mult)
            wB_ = scr.tile([128, yn], F32, tag="sc_wB", name="sc_wB")
            eng.tensor_tensor(wB_, fx, cm, OP.mult)
            eng.tensor_tensor(t1, g1, mhi, OP.mult)
            eng.tensor_tensor(wB_, wB_, t1, OP.add)
            eng.tensor_scalar(wB_, wB_, 0.25, None, OP.mult)
            return bcol, wA, wB_

        idf_halves = {}

        def phase_c_dve(half):
            ys = slice(half * 64, (half + 1) * 64)
            yn = 64
            bx, wL, wR = axis_pipeline(0, False, ys, yn)
            by, wT, wB_ = axis_pipeline(1, True, ys, yn)
            # packed k-minor weights; element layout is [TL | BL | TR | BR]
            v.tensor_tensor(wq4[:, ys, 0], wT, wL, OP.mult)
            v.tensor_tensor(wq4[:, ys, 1], wB_, wL, OP.mult)
            v.tensor_tensor(wq4[:, ys, 2], wT, wR, OP.mult)
            v.tensor_tensor(wq4[:, ys, 3], wB_, wR, OP.mult)
            # token index (y-major rows): idx = by*128 + bx.
            # bx/by are still in the u-domain (+384 each): subtract 384*129.
            idf = scr.tile([128, yn], F32, tag=f"sc_idf{half}", name="sc_idf")
            v.scalar_tensor_tensor(idf, by, 128.0, bx, OP.mult, OP.add)
            v.tensor_scalar(idf, idf, -384.0 * 129.0, None, OP.add)
            idf_halves[half] = idf

        def phase_c_sel(half):
            # Fold [x, y] -> gather layout [j%16, j//16] (j = y*128+x) with
            # the replication across the 8 16-partition groups baked in:
            #   ibT[p, y*8+g] = idf[g*16 + p%16, y]
            # via 8 selector matmuls (SELg[x, p] = 1 iff x == g*16 + p%16,
            # f32 exact) + strided-destination cast copies.  Compute-engine
            # APs must start at 32-aligned partitions, so the 16-row fold
            # cannot be done with plain copies.  Issued only once idf is
            # ready so they never head-of-line-block the PE queue.
            ys = slice(half * 64, (half + 1) * 64)
            yn = 64
            idf = idf_halves[half]
            for q in range(8):
                psS = psa.tile([128, 2, 256], F32, tag="psA", name="psS")
                nc.tensor.matmul(
                    psS[:, 0, 0:yn],
                    sel_sb[:, q * 128:(q + 1) * 128],
                    idf[:],
                    start=True,
                    stop=True,
                )
                v.tensor_copy(ibT_v[:, ys, q], psS[:, 0, 0:yn])

        # ---------------- Phase B: matmuls, y tokens ----------------
        # y rows are staged per 16-row batch plus the next batch's first row,
        # so each A-row [y_j | y_{j+128}] is written as one 512B run via an
        # overlapping-source DMA (row r and r+1 read twice).
        y_writes = [None] * n_xbat
        prev_yst = None

        def issue_ywrite(t, yst_t, nrows):
            src = AP(
                yst_t[:].tensor, yst_t[:].offset,
                [[(XB + 1) * C, 128], [C, nrows], [C, 2], [1, C]],
            )
            dst = yv4[t * XB * 128:(t * XB + nrows) * 128, :].rearrange(
                "(r x) d -> x r d", x=128
            )
            y_writes[t] = nc.sync.dma_start(dst, src)

        for t in range(n_xbat):
            yst = yp.tile([128, XB + 1, C], FP8, tag="yst", name="yst")
            psO = pso.tile([128, XB, 2], F32, tag="psO", name="psO")
            for hb in range(XB // 2):
                psA = psa.tile([128, 2, 256], F32, tag="psA", name="psA")
                for r in range(2):
                    cc = (hb * 2 + r) * 128
                    rr2 = hb * 2 + r
                    nc.tensor.matmul(
                        psA[:, r, 0:C], x_sb[t][:, 0, cc:cc + 128],
                        B_sb[0][:], start=True, stop=False,
                    )
                    nc.tensor.matmul(
                        psA[:, r, 0:C], x_sb[t][:, 1, cc:cc + 128],
                        B_sb[1][:], start=False, stop=True,
                    )
                    # raw offsets accumulate in a batch-persistent psum so
                    # they drain once per batch instead of once per pair
                    nc.tensor.matmul(
                        psO[:, rr2, :], x_sb[t][:, 0, cc:cc + 128],
                        woffb_sb[0][:], start=True, stop=False,
                        skip_group_check=True,
                    )
                    nc.tensor.matmul(
                        psO[:, rr2, :], x_sb[t][:, 1, cc:cc + 128],
                        woffb_sb[1][:], start=False, stop=True,
                        skip_group_check=True,
                    )
                rr = hb * 2
                # y tokens -> fp8 staging (ACT)
                s.copy(yst[:, rr:rr + 2, :], psA[:, :, 0:C])
                if hb == 0 and t > 0:
                    # previous batch needs this row as its 17th token row
                    s.copy(prev_yst[:, XB:XB + 1, :], psA[:, 0:1, 0:C])
                    issue_ywrite(t - 1, prev_yst, XB)
            # offsets: [x, r, o] -> offs[x, o, 16t+r], one drain per batch
            # (ACT: keeps the in-order DVE queue free for phase C)
            s.copy(
                offs_sb[:, :, t * XB:(t + 1) * XB].transpose([0, 2, 1]),
                psO[:],
            )
            prev_yst = yst
            if t == 3:
                phase_c_dve(0)
            elif t == 5:
                phase_c_sel(0)
        # last batch: 15 full A-rows (row 127's A-row is never gathered)
        issue_ywrite(n_xbat - 1, prev_yst, XB - 1)
        phase_c_dve(1)
        phase_c_sel(1)

        # ---------------- Phase D: gather + bilinear combine ----------------
        n_chunk = H // GC
        nidx = GC * 128
        for k in range(n_chunk):
            # rows this chunk can touch: by <= 8k+72 (clamped 126), +1 for
            # the second A-row of the element
            nrows = min(HW - 1, (GC * k + GC - 1 + 66) * 128)
            y2d = AP(
                y_dram[:].tensor, y_dram[:].offset, [[2 * C, nrows], [1, 4 * C]]
            )
            g4 = gp.tile([128, GC, 4 * C], FP8, tag="g4", name="g4")
            icols = slice(k * nidx // 16, (k + 1) * nidx // 16)
            gi = g.dma_gather(
                g4[:], y2d, ibT[:, icols], nidx, nidx,
                elem_size=4 * C, elem_step=2 * C,
            )
            add_dep_helper(gi.ins, lib_load.ins, reason="gather needs mlp lib")
            add_dep_helper(gi.ins, wz.ins, reason="gather window covers tail")
            bmax = min(n_xbat - 1, (GC * k + GC - 1 + 65) // XB)
            for wy in y_writes[:bmax + 1]:
                add_dep_helper(gi.ins, wy.ins, reason="gather reads y_dram")
            ost = [
                op_.tile([128, GC, 128], BF16, tag=f"ost{h}", name=f"ost{h}")
                for h in range(2)
            ]
            # diag tiles for all 4 corners of each row, built in one DVE op
            dgas = []
            for r in range(GC):
                y = k * GC + r
                dga = ap_.tile([128, 128, 4], BF16, tag="dga", name="dga")
                v.tensor_tensor(
                    dga[:], i4_sb[:],
                    wq4[:, y, :].unsqueeze(1).broadcast_to((128, 128, 4)),
                    OP.mult,
                )
                dgas.append(dga)
            for grp in range(GC // 4):
                psD = psd.tile([128, 2, 4, 128], F32, tag="psD", name="psD")
                row0 = k * GC + grp * 4
                t = row0 // XB
                coff = (row0 % XB) * 128
                for h in range(2):
                    # seed: psum = x; each seed covers its full psum bank
                    # exactly once (one open accumulation group per bank)
                    nc.tensor.matmul(
                        psD[:, h, :, :].rearrange("p r x -> p (r x)"),
                        id_sb[:],
                        x_sb[t][:, h, coff:coff + 512],
                        start=True, stop=False, skip_group_check=True,
                    )
                for r2 in range(4):
                    r = grp * 4 + r2
                    dga = dgas[r]
                    for h in range(2):
                        for q in range(4):
                            nc.tensor.matmul(
                                psD[:, h, r2, :],
                                g4[:, r, q * 256 + h * 128: q * 256 + h * 128 + 128],
                                dga[:, :, q],
                                start=False,
                                stop=(q == 3),
                                skip_group_check=True,
                            )
                for h in range(2):
                    s.activation(
                        ost[h][:, grp * 4:grp * 4 + 4, :],
                        psD[:, h, :, :],
                        AF.Identity,
                        bias=bwh_sb[:, h:h + 1],
                    )
            for h in range(2):
                ov = out_d.ap()[
                    h * 128:(h + 1) * 128, k * nidx:(k + 1) * nidx
                ].rearrange("p (r x) -> p r x", x=128)
                nc.sync.dma_start(ov, ost[h][:])


def _sel_const():
    sel = np.zeros((128, 8, 128), dtype=np.float32)
    for gq in range(8):
        for p in range(128):
            sel[gq * 16 + p % 16, gq, p] = 1.0
    return sel.reshape(128, 8 * 128)


def _host_inputs(inputs):
    """Per-core in_maps from the full problem inputs (layout/shard only)."""
    x = np.asarray(inputs["x"], dtype=np.float32)
    w_dw = np.asarray(inputs["w_dw"], dtype=np.float32)
    w_off = np.asarray(inputs["w_off"], dtype=np.float32)
    b_off = np.asarray(inputs["b_off"], dtype=np.float32)
    w_pc = np.asarray(inputs["w_pc"], dtype=np.float32)
    w_W = np.asarray(inputs["w_W"], dtype=np.float32)
    b_W = np.asarray(inputs["b_W"], dtype=np.float32)

    lin_w = np.linspace(-1.0, 1.0, W, dtype=np.float32)
    lin_h = np.linspace(-1.0, 1.0, H, dtype=np.float32)
    ident = np.eye(128, dtype=np.float32)
    shared = {
        "w_pc": np.ascontiguousarray(w_pc),
        "w_W_T": np.ascontiguousarray(w_W.T),
        "w_dw_p": np.ascontiguousarray(w_dw.reshape(2, 128).T) * 16.0,
        "w_off_T": np.ascontiguousarray(w_off.T),
        "b_off_b": b_off.reshape(1, 2),
        "bW_h": np.ascontiguousarray(b_W.reshape(2, 128).T),
        "gxb": lin_w.reshape(W, 1),
        "gyb": lin_h.reshape(1, H),
        "i4t": np.ascontiguousarray(np.repeat(ident, 4, axis=1)),
        "id128": ident,
        "sel": _sel_const(),
    }
    in_maps = []
    for b in range(B):
        m = dict(shared)
        m["x"] = np.ascontiguousarray(x[b].reshape(C, HW)).astype(NP_BF16)
        in_maps.append(m)
    return in_maps


def postprocess_core(raw):
    """Device out_cm [C, HW] bf16 -> [C, H, W] f32."""
    return np.asarray(raw).astype(np.float32).reshape(C, H, W)


def kernel_with_results(trace=False, **inputs):
    if "nc" not in _CACHE:
        _CACHE["nc"] = _build()
    nc = _CACHE["nc"]
    in_maps = _host_inputs(inputs)
    res = bass_utils.run_bass_kernel_spmd(
        nc, in_maps, core_ids=list(range(N_CORES)), trace=trace
    )
    outs = [postprocess_core(res.results[b]["out_cm"]) for b in range(B)]
    return np.stack(outs, axis=0), res


def kernel(**inputs) -> np.ndarray:
    out, _ = kernel_with_results(**inputs)
    return out


# revision 27
# speedup vs baseline: 1.3073x; 1.0629x over previous
"""CDConvBlock Trainium2 kernel (8-core SPMD, data-parallel over batch).

Math (per image, x: [C, H*W] channel-major):
    offset = tanh(w_off @ x + b_off)                      [2, HW]
    gx = clip(gx_base + offset[0], -1, 1), gy likewise
    A' = (w_W @ w_pc) * w_dw[None, :]                     [C, C]
    out = bilinear_zeros(A' @ x, gx, gy) + b_W + x
The two 1x1 convs and the depthwise scale commute with the per-channel
bilinear gather, so they fold into the single matrix A' applied BEFORE
sampling.

v2 pipeline (channel-major output; per core = one image):
  1. Stream x (cast to bf16, channel-major, kept resident in SBUF).
     For each 128-pixel row chunk, an x-stationary matmul (lhsT = x
     chunk) against rhs = [A'.T | w_off.T] yields a PIXEL-major psum
     [128px, 258] = [y row | dx_pre | dy_pre]; y rows are cast to fp8
     tokens and written TWICE to DRAM in a duplicated layout
     A[j] = [y_j | y_{j+128}]  (so one gather element = 4 corners).
  2. Offset math on small [128x, 128y] tiles -> bilinear base index
     idx = by*128+bx and 4 edge-corrected weights packed k-minor into
     wq4[x, y, k] (zeros padding folded into the weights).
  3. ONE dma_gather per 8-row chunk fetches 1 KB elements
     [TL|BL|TR|BR] (elem_step=512B overlapping, elem_size=1024B).
  4. Bilinear combine on PE, channel-major: per output row, psum
     [ch_half, px] is seeded with x via an identity matmul, then 4
     matmuls accumulate  G_q^T @ diag(w_q)  (diag tiles built in ONE
     DVE op per row from a replicated-identity constant).  ACT drains
     psum + b_W (per-partition bias) to bf16; result streams to DRAM
     channel-major [C, HW] -- the residual never needs a transpose.
Host only shards/reshapes (batch split, weight transposes, constants).
"""

import numpy as np

import concourse.bass as bass
import concourse.bacc as bacc
import concourse.mybir as mybir
from concourse.bass_types import AP
from concourse.tile import TileContext
from concourse.tile_rust import add_dep_helper
from concourse import bass_utils
from concourse import library_config

F32 = mybir.dt.float32
BF16 = mybir.dt.bfloat16
I16 = mybir.dt.int16
FP8 = mybir.dt.float8e4
NP_BF16 = mybir.dt.np(BF16)
OP = mybir.AluOpType
AF = mybir.ActivationFunctionType

B, C, H, W = 8, 256, 128, 128
HW = H * W
N_CORES = 8

XB = 16     # image rows per x-stream batch
GC = 8      # image rows per gather chunk

_CACHE = {}


def _build(reps=1):
    nc = bacc.Bacc(
        "TRN2", target_bir_lowering=False, debug=False, num_devices=N_CORES
    )
    x_d = nc.dram_tensor("x", [C, HW], BF16, kind="ExternalInput")
    wpc_d = nc.dram_tensor("w_pc", [C, C], F32, kind="ExternalInput")
    wwt_d = nc.dram_tensor("w_W_T", [C, C], F32, kind="ExternalInput")
    wdw_d = nc.dram_tensor("w_dw_p", [128, 2], F32, kind="ExternalInput")
    wofft_d = nc.dram_tensor("w_off_T", [C, 2], F32, kind="ExternalInput")
    boff_d = nc.dram_tensor("b_off_b", [1, 2], F32, kind="ExternalInput")
    bwh_d = nc.dram_tensor("bW_h", [128, 2], F32, kind="ExternalInput")
    gxb_d = nc.dram_tensor("gxb", [W, 1], F32, kind="ExternalInput")
    gyb_d = nc.dram_tensor("gyb", [1, H], F32, kind="ExternalInput")
    i4t_d = nc.dram_tensor("i4t", [128, 512], F32, kind="ExternalInput")
    id_d = nc.dram_tensor("id128", [128, 128], F32, kind="ExternalInput")
    sel_d = nc.dram_tensor("sel", [128, 8 * 128], F32, kind="ExternalInput")
    out_d = nc.dram_tensor("out_cm", [C, HW], BF16, kind="ExternalOutput")

    with TileContext(nc) as tc:
        for _ in range(reps):
            _kernel_body(
                nc, tc, x_d, wpc_d, wwt_d, wdw_d, wofft_d, boff_d, bwh_d,
                gxb_d, gyb_d, i4t_d, id_d, sel_d, out_d,
            )
    nc.finalize()
    return nc


def _kernel_body(
    nc, tc, x_d, wpc_d, wwt_d, wdw_d, wofft_d, boff_d, bwh_d,
    gxb_d, gyb_d, i4t_d, id_d, sel_d, out_d,
):
    v = nc.vector
    g = nc.gpsimd
    s = nc.scalar

    with (
        tc.tile_pool(name="persist", bufs=1) as pp,
        tc.tile_pool(name="wload", bufs=1) as wp,
        tc.tile_pool(name="scr", bufs=2) as scr,
        tc.tile_pool(name="yst", bufs=3) as yp,
        tc.tile_pool(name="gat", bufs=3) as gp,
        tc.tile_pool(name="dga", bufs=8) as ap_,
        tc.tile_pool(name="ost", bufs=2) as op_,
        tc.tile_pool(name="psA", bufs=2, space="PSUM") as psa,
        tc.tile_pool(name="psD", bufs=2, space="PSUM") as psd,
        tc.tile_pool(name="psO", bufs=2, space="PSUM") as pso,
        tc.tile_pool(name="dram", bufs=1, space="DRAM") as dp,
    ):
        # gpsimd ucode library for DMAGatherAnt
        lib_load = g.load_library(library_config.mlp)

        # ---------------- Phase A: weight prep ----------------
        wpc_sb = [wp.tile([128, C], F32, tag=f"wpc{m}", name=f"wpc{m}") for m in range(2)]
        wwt_sb = [wp.tile([128, C], F32, tag=f"wwt{m}", name=f"wwt{m}") for m in range(2)]
        wdw_sb = wp.tile([128, 2], F32, tag="wdw", name="wdw")
        wofft_sb = [wp.tile([128, 2], F32, tag=f"woft{m}", name=f"woft{m}") for m in range(2)]
        for m in range(2):
            nc.sync.dma_start(wpc_sb[m][:], wpc_d.ap()[m * 128:(m + 1) * 128, :])
            nc.sync.dma_start(wwt_sb[m][:], wwt_d.ap()[m * 128:(m + 1) * 128, :])
            nc.sync.dma_start(
                wofft_sb[m][:], wofft_d.ap()[m * 128:(m + 1) * 128, :]
            )
        nc.sync.dma_start(wdw_sb[:], wdw_d.ap())

        boff_sb = pp.tile([128, 2], F32, tag="boff", name="boff")
        bwh_sb = pp.tile([128, 2], F32, tag="bwh", name="bwh")
        gxb_sb = pp.tile([128, 1], F32, tag="gxb", name="gxb")
        gyb_sb = pp.tile([128, H], F32, tag="gyb", name="gyb")
        nc.sync.dma_start(boff_sb[:], boff_d.ap().to_broadcast((128, 2)))
        nc.sync.dma_start(bwh_sb[:], bwh_d.ap())
        nc.sync.dma_start(gxb_sb[:], gxb_d.ap())
        nc.sync.dma_start(gyb_sb[:], gyb_d.ap().to_broadcast((128, H)))

        i4_sb = pp.tile([128, 128, 4], BF16, tag="i4", name="i4")
        g.dma_start(i4_sb[:].rearrange("p j k -> p (j k)"), i4t_d.ap())
        id_sb = pp.tile([128, 128], BF16, tag="id", name="id")
        g.dma_start(id_sb[:], id_d.ap())
        sel_sb = pp.tile([128, 8 * 128], F32, tag="sel", name="sel")
        nc.sync.dma_start(sel_sb[:], sel_d.ap())

        # persistent x (bf16, channel-major), 8 column-batches per half.
        # All loads issued up front so later y writes (whose deps resolve
        # mid-phase-B) never head-of-line-block them on the sync queue.
        n_xbat = H // XB
        x_sb = [
            pp.tile([128, 2, XB * 128], BF16, tag=f"x_{t}", name=f"x_{t}")
            for t in range(n_xbat)
        ]
        for t in range(n_xbat):
            cols = slice(t * XB * 128, (t + 1) * XB * 128)
            nc.sync.dma_start(
                x_sb[t][:],
                x_d.ap()[:, cols].rearrange("(h p) c -> p h c", h=2),
            )

        # B matrices: B[cb] = [A'.T | w_off.T] rows cb*128..cb*128+128
        B_sb = [pp.tile([128, C], BF16, tag=f"B{cb}", name=f"B{cb}") for cb in range(2)]
        woffb_sb = [
            pp.tile([128, 2], BF16, tag=f"wofb{m}", name=f"wofb{m}")
            for m in range(2)
        ]
        for cb in range(2):
            ps = psa.tile([128, 2, 256], F32, tag="psA", name="psA")
            for m in range(2):
                nc.tensor.matmul(
                    ps[:, 0, 0:C],
                    wpc_sb[m][:, cb * 128:(cb + 1) * 128],
                    wwt_sb[m][:],
                    start=(m == 0),
                    stop=(m == 1),
                )
            v.tensor_scalar(
                B_sb[cb][:, 0:C], ps[:, 0, 0:C], wdw_sb[:, cb:cb + 1], None,
                OP.mult,
            )
            v.tensor_copy(woffb_sb[cb][:], wofft_sb[cb][:])

        offs_sb = pp.tile([128, 2, H], F32, tag="offs", name="offs")    # [x, {dx,dy}, y]
        wq4 = pp.tile([128, H, 4], F32, tag="wq4", name="wq4")
        ibT = pp.tile([128, HW // 16], I16, tag="ibT", name="ibT")

        # y-dup token array: A[j] = [y_j | y_{j+128}], 512B rows
        y_dram = dp.tile([HW * 2 * C], FP8, tag="ydram", name="ydram")
        yv4 = y_dram[:].rearrange("(j d) -> j d", d=2 * C)
        # zero the never-read last image row so the finite-guard on the
        # gather's full address window passes
        zt = pp.tile([128, 2 * C], FP8, tag="zt", name="zt")
        v.memzero(zt[:])
        wz = nc.sync.dma_start(
            yv4[HW - 128:HW, :].rearrange("(r x) d -> x r d", x=128),
            zt[:].unsqueeze(1),
        )

        # ---------------- Phase C body (per y-column half) ----------------
        ibT_v = ibT[:].rearrange("p (y q) -> p y q", q=8)

        def axis_pipeline(o, base_is_tensor, ys, yn, eng=v):
            """offs[:, o, ys] -> (b (f32 col base), wA, wB) for one axis."""
            d = scr.tile([128, yn], F32, tag="sc_d", name="sc_d")
            if base_is_tensor:
                s.activation(d, offs_sb[:, o, ys], AF.Tanh, bias=boff_sb[:, o:o + 1])
                gg = scr.tile([128, yn], F32, tag="sc_g", name="sc_g")
                eng.scalar_tensor_tensor(gg, d, 1.0, gyb_sb[:, ys], OP.mult, OP.add)
            else:
                s.activation(d, offs_sb[:, o, ys], AF.Tanh, bias=boff_sb[:, o:o + 1])
                gg = scr.tile([128, yn], F32, tag="sc_g", name="sc_g")
                eng.tensor_scalar(gg, d, gxb_sb[:, 0:1], None, OP.add)
            # u = ix + 384 lives in the f32 binade [256, 512) where ulp is
            # 2^-15, so clearing the low 15 mantissa bits IS floor(u) for
            # the whole range ix in [-0.5, 127.5].  All exact.
            ix = scr.tile([128, yn], F32, tag="sc_ix", name="sc_ix")
            eng.tensor_scalar(ix, gg, -1.0, 1.0, OP.max, OP.min)
            eng.tensor_scalar(ix, ix, 64.0, 63.5 + 384.0, OP.mult, OP.add)
            ufl = scr.tile([128, yn], F32, tag="sc_ufl", name="sc_ufl")
            eng.tensor_scalar(
                ufl[:].bitcast(mybir.dt.uint32), ix[:].bitcast(mybir.dt.uint32),
                0xFFFF8000, None, OP.bitwise_and,
            )
            fx = scr.tile([128, yn], F32, tag="sc_fx", name="sc_fx")
            eng.tensor_tensor(fx, ix, ufl, OP.subtract)
            mneg = scr.tile([128, yn], F32, tag="sc_mneg", name="sc_mneg")
            eng.tensor_scalar(mneg, ufl, 383.5, None, OP.is_lt)
            mhi = scr.tile([128, yn], F32, tag="sc_mhi", name="sc_mhi")
            eng.tensor_scalar(mhi, ufl, 510.5, None, OP.is_gt)
            # clamped base, still in u-domain (bx + 384)
            bcol = scr.tile([128, yn], F32, tag="sc_b", name="sc_b")
            eng.tensor_scalar(bcol, ufl, 384.0, 510.0, OP.max, OP.min)
            # common = 1 - mneg - mhi;  g1 = 1 - fx
            cm = scr.tile([128, yn], F32, tag="sc_cm", name="sc_cm")
            eng.tensor_tensor(cm, mneg, mhi, OP.add)
            eng.tensor_scalar(cm, cm, -1.0, 1.0, OP.mult, OP.add)
            g1 = scr.tile([128, yn], F32, tag="sc_g1", name="sc_g1")
            eng.tensor_scalar(g1, fx, -1.0, 1.0, OP.mult, OP.add)
            # wA = g1*common + fx*mneg ; wB = fx*common + g1*mhi
            wA = scr.tile([128, yn], F32, tag="sc_wA", name="sc_wA")
            t1 = scr.tile([128, yn], F32, tag="sc_t1", name="sc_t1")
            eng.tensor_tensor(wA, g1, cm, OP.mult)
            eng.tensor_tensor(t1, fx, mneg, OP.mult)
            eng.tensor_tensor(wA, wA, t1, OP.add)
            eng.tensor_scalar(wA, wA, 0.25, None, OP.mult)
            wB_ = scr.tile([128, yn], F32, tag="sc_wB", name="sc_wB")
            eng.tensor_tensor(wB_, fx, cm, OP.mult)
            eng.tensor_tensor(t1, g1, mhi, OP.mult)
            eng.tensor_tensor(wB_, wB_, t1, OP.add)
            eng.tensor_scalar(wB_, wB_, 0.25, None, OP.mult)
            return bcol, wA, wB_

        idf_halves = {}

        def phase_c_dve(half):
            ys = slice(half * 64, (half + 1) * 64)
            yn = 64
            bx, wL, wR = axis_pipeline(0, False, ys, yn)
            by, wT, wB_ = axis_pipeline(1, True, ys, yn)
            # packed k-minor weights; element layout is [TL | BL | TR | BR]
            v.tensor_tensor(wq4[:, ys, 0], wT, wL, OP.mult)
            v.tensor_tensor(wq4[:, ys, 1], wB_, wL, OP.mult)
            v.tensor_tensor(wq4[:, ys, 2], wT, wR, OP.mult)
            v.tensor_tensor(wq4[:, ys, 3], wB_, wR, OP.mult)
            # token index (y-major rows): idx = by*128 + bx.
            # bx/by are still in the u-domain (+384 each): subtract 384*129.
            idf = scr.tile([128, yn], F32, tag=f"sc_idf{half}", name="sc_idf")
            v.scalar_tensor_tensor(idf, by, 128.0, bx, OP.mult, OP.add)
            v.tensor_scalar(idf, idf, -384.0 * 129.0, None, OP.add)
            idf_halves[half] = idf

        def phase_c_sel(half):
            # Fold [x, y] -> gather layout [j%16, j//16] (j = y*128+x) with
            # the replication across the 8 16-partition groups baked in:
            #   ibT[p, y*8+g] = idf[g*16 + p%16, y]
            # via 8 selector matmuls (SELg[x, p] = 1 iff x == g*16 + p%16,
            # f32 exact) + strided-destination cast copies.  Compute-engine
            # APs must start at 32-aligned partitions, so the 16-row fold
            # cannot be done with plain copies.  Issued only once idf is
            # ready so they never head-of-line-block the PE queue.
            ys = slice(half * 64, (half + 1) * 64)
            yn = 64
            idf = idf_halves[half]
            for q in range(8):
                psS = psa.tile([128, 2, 256], F32, tag="psA", name="psS")
                nc.tensor.matmul(
                    psS[:, 0, 0:yn],
                    sel_sb[:, q * 128:(q + 1) * 128],
                    idf[:],
                    start=True,
                    stop=True,
                )
                v.tensor_copy(ibT_v[:, ys, q], psS[:, 0, 0:yn])

        # ---------------- Phase B: matmuls, y tokens ----------------
        # y rows are staged per 16-row batch plus the next batch's first row,
        # so each A-row [y_j | y_{j+128}] is written as one 512B run via an
        # overlapping-source DMA (row r and r+1 read twice).
        y_writes = [None] * n_xbat
        prev_yst = None

        def issue_ywrite(t, yst_t, nrows):
            src = AP(
                yst_t[:].tensor, yst_t[:].offset,
                [[(XB + 1) * C, 128], [C, nrows], [C, 2], [1, C]],
            )
            dst = yv4[t * XB * 128:(t * XB + nrows) * 128, :].rearrange(
                "(r x) d -> x r d", x=128
            )
            y_writes[t] = nc.sync.dma_start(dst, src)

        for t in range(n_xbat):
            yst = yp.tile([128, XB + 1, C], FP8, tag="yst", name="yst")
            psO = pso.tile([128, XB, 2], F32, tag="psO", name="psO")
            for hb in range(XB // 2):
                psA = psa.tile([128, 2, 256], F32, tag="psA", name="psA")
                for r in range(2):
                    cc = (hb * 2 + r) * 128
                    rr2 = hb * 2 + r
                    nc.tensor.matmul(
                        psA[:, r, 0:C], x_sb[t][:, 0, cc:cc + 128],
                        B_sb[0][:], start=True, stop=False,
                    )
                    nc.tensor.matmul(
                        psA[:, r, 0:C], x_sb[t][:, 1, cc:cc + 128],
                        B_sb[1][:], start=False, stop=True,
                    )
                    # raw offsets accumulate in a batch-persistent psum so
                    # they drain once per batch instead of once per pair
                    nc.tensor.matmul(
                        psO[:, rr2, :], x_sb[t][:, 0, cc:cc + 128],
                        woffb_sb[0][:], start=True, stop=False,
                        skip_group_check=True,
                    )
                    nc.tensor.matmul(
                        psO[:, rr2, :], x_sb[t][:, 1, cc:cc + 128],
                        woffb_sb[1][:], start=False, stop=True,
                        skip_group_check=True,
                    )
                rr = hb * 2
                # y tokens -> fp8 staging (ACT)
                s.copy(yst[:, rr:rr + 2, :], psA[:, :, 0:C])
                if hb == 0 and t > 0:
                    # previous batch needs this row as its 17th token row
                    s.copy(prev_yst[:, XB:XB + 1, :], psA[:, 0:1, 0:C])
                    issue_ywrite(t - 1, prev_yst, XB)
            # offsets: [x, r, o] -> offs[x, o, 16t+r], one drain per batch
            # (ACT: keeps the in-order DVE queue free for phase C)
            s.copy(
                offs_sb[:, :, t * XB:(t + 1) * XB].transpose([0, 2, 1]),
                psO[:],
            )
            prev_yst = yst
            if t == 3:
                phase_c_dve(0)
            elif t == 4:
                phase_c_sel(0)
        # last batch: 15 full A-rows (row 127's A-row is never gathered)
        issue_ywrite(n_xbat - 1, prev_yst, XB - 1)
        phase_c_dve(1)
        phase_c_sel(1)

        # ---------------- Phase D: gather + bilinear combine ----------------
        n_chunk = H // GC
        nidx = GC * 128
        for k in range(n_chunk):
            # rows this chunk can touch: by <= 8k+72 (clamped 126), +1 for
            # the second A-row of the element
            nrows = min(HW - 1, (GC * k + GC - 1 + 66) * 128)
            y2d = AP(
                y_dram[:].tensor, y_dram[:].offset, [[2 * C, nrows], [1, 4 * C]]
            )
            g4 = gp.tile([128, GC, 4 * C], FP8, tag="g4", name="g4")
            icols = slice(k * nidx // 16, (k + 1) * nidx // 16)
            gi = g.dma_gather(
                g4[:], y2d, ibT[:, icols], nidx, nidx,
                elem_size=4 * C, elem_step=2 * C,
            )
            add_dep_helper(gi.ins, lib_load.ins, reason="gather needs mlp lib")
            add_dep_helper(gi.ins, wz.ins, reason="gather window covers tail")
            bmax = min(n_xbat - 1, (GC * k + GC - 1 + 65) // XB)
            for wy in y_writes[:bmax + 1]:
                add_dep_helper(gi.ins, wy.ins, reason="gather reads y_dram")
            ost = [
                op_.tile([128, GC, 128], BF16, tag=f"ost{h}", name=f"ost{h}")
                for h in range(2)
            ]
            # diag tiles for all 4 corners of each row, built in one DVE op
            dgas = []
            for r in range(GC):
                y = k * GC + r
                dga = ap_.tile([128, 128, 4], FP8, tag="dga", name="dga")
                v.tensor_tensor(
                    dga[:], i4_sb[:],
                    wq4[:, y, :].unsqueeze(1).broadcast_to((128, 128, 4)),
                    OP.mult,
                )
                dgas.append(dga)
            for grp in range(GC // 4):
                psD = psd.tile([128, 2, 4, 128], F32, tag="psD", name="psD")
                row0 = k * GC + grp * 4
                t = row0 // XB
                coff = (row0 % XB) * 128
                for h in range(2):
                    # seed: psum = x; each seed covers its full psum bank
                    # exactly once (one open accumulation group per bank)
                    nc.tensor.matmul(
                        psD[:, h, :, :].rearrange("p r x -> p (r x)"),
                        id_sb[:],
                        x_sb[t][:, h, coff:coff + 512],
                        start=True, stop=False, skip_group_check=True,
                    )
                for r2 in range(4):
                    r = grp * 4 + r2
                    dga = dgas[r]
                    g4r = g4[:, r, :].rearrange("p (q c) -> p q c", c=256)
                    for h in range(2):
                        for pair in range(2):
                            # fp8 DoubleRow: one matmul contracts both
                            # corner terms q = 2*pair, 2*pair+1
                            nc.tensor.matmul(
                                psD[:, h, r2, :],
                                g4r[:, 2 * pair:2 * pair + 2,
                                    h * 128:h * 128 + 128],
                                dga[:, :, 2 * pair:2 * pair + 2].transpose(
                                    [0, 2, 1]
                                ),
                                start=False,
                                stop=(pair == 1),
                                perf_mode=mybir.MatmulPerfMode.DoubleRow,
                                skip_group_check=True,
                            )
                for h in range(2):
                    s.activation(
                        ost[h][:, grp * 4:grp * 4 + 4, :],
                        psD[:, h, :, :],
                        AF.Identity,
                        bias=bwh_sb[:, h:h + 1],
                    )
            for h in range(2):
                ov = out_d.ap()[
                    h * 128:(h + 1) * 128, k * nidx:(k + 1) * nidx
                ].rearrange("p (r x) -> p r x", x=128)
                nc.sync.dma_start(ov, ost[h][:])


def _sel_const():
    sel = np.zeros((128, 8, 128), dtype=np.float32)
    for gq in range(8):
        for p in range(128):
            sel[gq * 16 + p % 16, gq, p] = 1.0
    return sel.reshape(128, 8 * 128)


def _host_inputs(inputs):
    """Per-core in_maps from the full problem inputs (layout/shard only)."""
    x = np.asarray(inputs["x"], dtype=np.float32)
    w_dw = np.asarray(inputs["w_dw"], dtype=np.float32)
    w_off = np.asarray(inputs["w_off"], dtype=np.float32)
    b_off = np.asarray(inputs["b_off"], dtype=np.float32)
    w_pc = np.asarray(inputs["w_pc"], dtype=np.float32)
    w_W = np.asarray(inputs["w_W"], dtype=np.float32)
    b_W = np.asarray(inputs["b_W"], dtype=np.float32)

    lin_w = np.linspace(-1.0, 1.0, W, dtype=np.float32)
    lin_h = np.linspace(-1.0, 1.0, H, dtype=np.float32)
    ident = np.eye(128, dtype=np.float32)
    shared = {
        "w_pc": np.ascontiguousarray(w_pc),
        "w_W_T": np.ascontiguousarray(w_W.T),
        "w_dw_p": np.ascontiguousarray(w_dw.reshape(2, 128).T) * 16.0,
        "w_off_T": np.ascontiguousarray(w_off.T),
        "b_off_b": b_off.reshape(1, 2),
        "bW_h": np.ascontiguousarray(b_W.reshape(2, 128).T),
        "gxb": lin_w.reshape(W, 1),
        "gyb": lin_h.reshape(1, H),
        "i4t": np.ascontiguousarray(np.repeat(ident, 4, axis=1)),
        "id128": ident,
        "sel": _sel_const(),
    }
    in_maps = []
    for b in range(B):
        m = dict(shared)
        m["x"] = np.ascontiguousarray(x[b].reshape(C, HW)).astype(NP_BF16)
        in_maps.append(m)
    return in_maps


def postprocess_core(raw):
    """Device out_cm [C, HW] bf16 -> [C, H, W] f32."""
    return np.asarray(raw).astype(np.float32).reshape(C, H, W)


def kernel_with_results(trace=False, **inputs):
    if "nc" not in _CACHE:
        _CACHE["nc"] = _build()
    nc = _CACHE["nc"]
    in_maps = _host_inputs(inputs)
    res = bass_utils.run_bass_kernel_spmd(
        nc, in_maps, core_ids=list(range(N_CORES)), trace=trace
    )
    outs = [postprocess_core(res.results[b]["out_cm"]) for b in range(B)]
    return np.stack(outs, axis=0), res


def kernel(**inputs) -> np.ndarray:
    out, _ = kernel_with_results(**inputs)
    return out
